# revision 37
# baseline (speedup 1.0000x reference)
"""Trainium2 Bass kernel for nn_GPTrack2D (dense transformer with linear
attention and a per-frame recurrence over L).

Sharding: batch (2) -> two groups of 4 cores; tokens (1024 -> 256/core)
within each group. Linear attention's k^T v state is all-reduced per frame
within the group; the all-reduce hides behind the previous frame's MLP and
the next frame's x-side LayerNorm (software-pipelined emission).

v3 over v2: fp8(e4m3) DoubleRow matmuls where numerics allow.
- q,k projections: fp8 DR pairing the x-side and h-side contractions
  (zz tile holds fp8 zx/zh planes; wqk8 holds both weight planes, x64).
- v projection stays f16 (fp8 v decorrelates the recurrence: sim showed
  6.8% rel err with v fp8 vs 0.57% with v f16).
- MLP: both GEMMs fp8 DR (g1 pairs kd, w2 pairs mj); numerically free.
- elu+1 on a x64-scaled psum via  max(x,0) + min(exp(x),1)  keeps the
  3-op structure with the 1/64 descale folded into ScalarE's scale arg.
- B2 bias rows padded 33->65 (rank-33 matmuls measured 237ns vs 134).
- Identity copies moved from ScalarE to DVE (ACT_TABLE_LOAD thrash:
  1283ns per function switch, was ~21/frame).
- wqk8/wv16 double-buffered; next segment prefetched mid-scan on the
  gpsimd queue, so scan boundaries don't stall on weight DMA.
- g1 fully SBUF-resident in fp8 (no streamed half).

Precision: residual stream / carry f32; LN stat inputs bf16 (|h| reaches
~1.3e6). kv state scaled by 1/256 to fit f16; wout pre-scaled by 256.
fp8 weights pre-scaled by 64 (0.02-scale weights underflow e4m3 norms).
"""

import functools

import numpy as np
import ml_dtypes

import concourse.bacc as bacc
import concourse.mybir as mybir
from concourse import tile
from concourse.bass_utils import run_bass_kernel_spmd

F32 = mybir.dt.float32
BF16 = mybir.dt.bfloat16
F16 = mybir.dt.float16
F8 = mybir.dt.float8e4
NP_F8 = ml_dtypes.float8_e4m3
AF = mybir.ActivationFunctionType
ALU = mybir.AluOpType

B, L, N, D, M, H = 2, 12, 1024, 768, 3072, 12
NCORES = 8
GROUP = 4                 # cores per batch group
TOK = N // GROUP          # 256 tokens per core
KT = D // 128             # 6 d-tiles
MT = M // 128             # 24 m-tiles
F3 = 3 * D                # 2304
EPS = 1e-5
KVS = 1.0 / 256.0         # kv-state scale so fp16 holds it
KVSI = 256.0
W8S = 64.0                # fp8 weight pre-scale
W8SI = 1.0 / 64.0

# dev-scale knobs (full problem: L_RUN=12, LAYERS_RUN=2, DIRS_RUN=(0, 1))
L_RUN = L
LAYERS_RUN = 2
DIRS_RUN = (0, 1)

REPLICA_GROUPS = [[0, 1, 2, 3], [4, 5, 6, 7]]


# ---------------------------------------------------------------- host prep

def _pack_weights(inputs):
    segs = []
    for layer in range(LAYERS_RUN):
        for d in DIRS_RUN:
            gi = np.asarray(inputs["lni_g"][d, layer]); bi = np.asarray(inputs["lni_b"][d, layer])
            gh = np.asarray(inputs["lnh_g"][d, layer]); bh = np.asarray(inputs["lnh_b"][d, layer])
            go = np.asarray(inputs["lno_g"][d, layer]); bo = np.asarray(inputs["lno_b"][d, layer])
            Wqkv = np.asarray(inputs["Wqkv"][d, layer]); bqkv = np.asarray(inputs["bqkv"][d, layer])
            Wqkvh = np.asarray(inputs["Wqkvh"][d, layer]); bqkvh = np.asarray(inputs["bqkvh"][d, layer])
            Wout = np.asarray(inputs["Wout"][d, layer]); bout = np.asarray(inputs["bout"][d, layer])
            W1 = np.asarray(inputs["W1"][d, layer]); b1 = np.asarray(inputs["b1"][d, layer])
            W2 = np.asarray(inputs["W2"][d, layer]); b2 = np.asarray(inputs["b2"][d, layer])

            gqkv = gi[:, None] * Wqkv                      # (D, 3D)
            gqkvh = gh[:, None] * Wqkvh
            cqkv = bi @ Wqkv + bqkv + bh @ Wqkvh + bqkvh   # (3D,)
            # [p, side, kd, f]: side 0 = x-weights, 1 = h-weights
            wpair = np.stack([
                gqkv.reshape(KT, 128, F3).transpose(1, 0, 2),
                gqkvh.reshape(KT, 128, F3).transpose(1, 0, 2)], axis=1)
            # q,k columns [0, 2D) in fp8 x64; v columns [2D, 3D) in f16
            wqk8 = np.ascontiguousarray(wpair[:, :, :, :2 * D] * W8S).astype(NP_F8)
            wv16 = np.ascontiguousarray(wpair[:, :, :, 2 * D:]).astype(np.float16)
            # bias rows on partitions 0/32/64 (DVE base-partition rule)
            B3 = np.zeros((65, F3), np.float32)
            B3[0], B3[32], B3[64] = cqkv, -gqkv.sum(0), -gqkvh.sum(0)
            B3qk = (B3[:, :2 * D] * W8S).astype(np.float16)
            B3v = B3[:, 2 * D:].astype(np.float16)

            g1 = go[:, None] * W1                          # (D, M)
            c1 = bo @ W1 + b1                              # (M,)
            B2 = np.zeros((65, M), np.float32)
            B2[0], B2[32] = c1 * W8S, -g1.sum(0) * W8S

            seg = dict(
                wqk8=wqk8, wv16=wv16, B3qk=B3qk, B3v=B3v,
                # wout pre-scaled by KVSI: cancels the f16 kv-state 1/256.
                # ft-chunked for streaming: [ft, p, kd, c] = w[kd*128+p, ft*128+c]
                wout=np.ascontiguousarray(
                    (Wout * KVSI).reshape(KT, 128, KT, 128)
                    .transpose(2, 1, 0, 3)).astype(np.float16),
                bout=np.ascontiguousarray(
                    bout.reshape(KT, 128).T).astype(np.float32),
                # fully resident: [p, mj, kd, c] = g1[kd*128+p, mj*128+c]
                g1=np.ascontiguousarray(
                    (g1 * W8S).reshape(KT, 128, MT, 128)
                    .transpose(1, 2, 0, 3)).astype(NP_F8),
                B2=B2.astype(np.float16),
                # mj-paired for DR: [mjp, p, i, f] = W2[(2mjp+i)*128+p, f]
                w2p=np.ascontiguousarray(
                    (W2 * W8S).reshape(MT // 2, 2, 128, D)
                    .transpose(0, 2, 1, 3)).astype(NP_F8),
                b2=np.ascontiguousarray(
                    b2.reshape(KT, 128).T).astype(np.float32),    # (128, KT)
            )
            segs.append(seg)
    return segs


def _feat_major(a, dtype):
    """(..., tok, D) -> (..., 128, KT, tok) tiled feature-major."""
    t = np.moveaxis(np.asarray(a), -1, -2)                # (..., D, tok)
    shp = t.shape[:-2]
    t = t.reshape(shp + (KT, 128, t.shape[-1]))           # (..., KT, 128, tok)
    t = np.moveaxis(t, -3, -2)                            # (..., 128, KT, tok)
    return np.ascontiguousarray(t).astype(dtype)


def make_in_maps(inputs):
    segs = _pack_weights(inputs)
    in_maps = []
    for core in range(NCORES):
        b = core // GROUP
        s = (core % GROUP) * TOK
        m = {}
        m["x_in"] = _feat_major(
            np.asarray(inputs["x"])[b, :L_RUN, s:s + TOK, :], np.float32)
        m["h0_in"] = _feat_major(
            np.asarray(inputs["hidden"])[b, s:s + TOK, :], np.float32)
        m["spat"] = _feat_major(
            np.asarray(inputs["spatial_pos"])[b, s:s + TOK, :], np.float32)
        tp = np.asarray(inputs["temporal_pos"])[b, :L_RUN, :]   # (L, D)
        tp = tp.T.reshape(KT, 128, L_RUN).transpose(1, 0, 2)
        m["tpos"] = np.ascontiguousarray(tp).astype(np.float32)  # (128, KT, L)
        for si, seg in enumerate(segs):
            for k, v in seg.items():
                m[f"{k}_{si}"] = v
        in_maps.append(m)
    return in_maps


def unshard_output(results):
    out = np.empty((B, L_RUN, N, D), np.float32)
    for core in range(NCORES):
        b = core // GROUP
        s = (core % GROUP) * TOK
        o = np.asarray(results[core]["out_x"])            # (L, 128, KT, TOK)
        o = o.transpose(0, 2, 1, 3).reshape(L_RUN, D, TOK)
        out[b, :, s:s + TOK, :] = np.moveaxis(o, -1, -2)
    return out


# ---------------------------------------------------------------- kernel build

class Ctx:
    """Pools, constants and persistent tiles used during emission."""

    ps_ctr = 0

    def ps(self):
        """[128, 512] f32 psum tile; every 3rd borrows the stats bank
        (idle between LN chains) so elu/copy latency never starves the
        2-buffer psA rotation."""
        self.ps_ctr += 1
        if self.ps_ctr % 3 == 0:
            return self.psS.tile([128, 2 * TOK], F32, name="s12", tag="s12")
        return self.psA.tile([128, 2 * TOK], F32, name="ps", tag="ps")


def _ln_chain(nc, cx, src32, tag, veng=None, buf_tag=None):
    """Feature-major LN for an SBUF (128, KT, TOK) f32 tile.

    Emits: bf16 copy xb, squares, packed stats matmuls (s1|s2 in one
    PSUM bank), mean/var smalls, sqrt + fast reciprocal. Returns (xb,
    rbb, mean, rb32). veng places the wide copy/square ops (gpsimd for
    off-critical chains keeps the DVE queue short).
    """
    veng = veng or nc.vector
    # xq packs [bf16 copy | its square] so one 512-wide matmul per kd
    # yields both stat sums; squares off ScalarE keep its LUT unthrashed
    xq = cx.act1.tile([128, KT, 2 * TOK], BF16, name=f"xb_{tag}",
                      tag=buf_tag or f"xb_{tag}")
    s12 = cx.psS.tile([128, 2 * TOK], F32, name="s12", tag="s12")
    for kd in range(KT):
        veng.tensor_copy(xq[:, kd, 0:TOK], src32[:, kd, :])
        veng.tensor_mul(xq[:, kd, TOK:2 * TOK], xq[:, kd, 0:TOK],
                        xq[:, kd, 0:TOK])
        nc.tensor.matmul(s12[:], cx.onesB[:], xq[:, kd, :],
                         start=(kd == 0), stop=(kd == KT - 1))
    mean = cx.sm.tile([128, TOK], F32, name="mean", tag="lnsm")
    nc.vector.tensor_scalar_mul(mean[:], s12[:, 0:TOK], 1.0 / D)
    msq = cx.sm.tile([128, TOK], F32, name="msq", tag="lnsm")
    nc.vector.tensor_mul(msq[:], mean[:], mean[:])
    ve = cx.sm.tile([128, TOK], F32, name="ve", tag="lnsm")
    nc.vector.scalar_tensor_tensor(ve[:], s12[:, TOK:2 * TOK], 1.0 / D, msq[:],
                                   op0=ALU.mult, op1=ALU.subtract)
    rb32 = cx.sm.tile([128, TOK], F32, name="rb32", tag="lnsm")
    # 1/sqrt(|ve|+eps) in one ScalarE op (Rsqrt is banned; this isn't)
    nc.scalar.activation(rb32[:], ve[:], AF.Abs_reciprocal_sqrt,
                         bias=cx.epsc[:])
    rbb = cx.tmp.tile([128, TOK], BF16, name=f"rbb_{tag}", tag=f"rbb_{tag}")
    nc.vector.tensor_copy(rbb[:], rb32[:])
    return xq, rbb, mean, rb32


def _elu1(nc, cx, psum_ap, out_ap, ncols):
    """out = elu(x)+1 where psum holds 64*x.

    max(x,0) + min(exp(x),1): tlin and texp both read the psum directly
    (no serial DVE->ScalarE chain); ScalarE folds the 1/64 descale.
    """
    tlin = cx.act1.tile([128, 512], F32, name="elin", tag="elin")
    texp = cx.act1.tile([128, 512], F32, name="eexp", tag="eexp")
    nc.vector.tensor_scalar(tlin[:, :ncols], psum_ap, W8SI, 0.0,
                            op0=ALU.mult, op1=ALU.max)
    nc.scalar.activation(texp[:, :ncols], psum_ap, AF.Exp, scale=W8SI)
    nc.vector.scalar_tensor_tensor(out_ap, texp[:, :ncols], 1.0,
                                   tlin[:, :ncols], op0=ALU.min, op1=ALU.add)


def build_nc():
    nc = bacc.Bacc("TRN2", target_bir_lowering=False, debug=False,
                   num_devices=NCORES)

    x_in = nc.dram_tensor("x_in", [L_RUN, 128, KT, TOK], F32, kind="ExternalInput")
    h0_in = nc.dram_tensor("h0_in", [128, KT, TOK], F32, kind="ExternalInput")
    spat = nc.dram_tensor("spat", [128, KT, TOK], F32, kind="ExternalInput")
    tpos = nc.dram_tensor("tpos", [128, KT, L_RUN], F32, kind="ExternalInput")
    nseg = LAYERS_RUN * len(DIRS_RUN)
    segs = []
    for si in range(nseg):
        segs.append(dict(
            wqk8=nc.dram_tensor(f"wqk8_{si}", [128, 2, KT, 2 * D], F8, kind="ExternalInput"),
            wv16=nc.dram_tensor(f"wv16_{si}", [128, 2, KT, D], F16, kind="ExternalInput"),
            B3qk=nc.dram_tensor(f"B3qk_{si}", [65, 2 * D], F16, kind="ExternalInput"),
            B3v=nc.dram_tensor(f"B3v_{si}", [65, D], F16, kind="ExternalInput"),
            wout=nc.dram_tensor(f"wout_{si}", [KT, 128, KT, 128], F16, kind="ExternalInput"),
            bout=nc.dram_tensor(f"bout_{si}", [128, KT], F32, kind="ExternalInput"),
            g1=nc.dram_tensor(f"g1_{si}", [128, MT, KT, 128], F8, kind="ExternalInput"),
            B2=nc.dram_tensor(f"B2_{si}", [65, M], F16, kind="ExternalInput"),
            w2p=nc.dram_tensor(f"w2p_{si}", [MT // 2, 128, 2, D], F8, kind="ExternalInput"),
            b2=nc.dram_tensor(f"b2_{si}", [128, KT], F32, kind="ExternalInput"),
        ))
    out_x = nc.dram_tensor("out_x", [L_RUN, 128, KT, TOK], F32, kind="ExternalOutput")

    with tile.TileContext(nc) as tc:
        with (
            tc.tile_pool(name="cst", bufs=1) as cst,
            tc.tile_pool(name="wt", bufs=2) as wt,
            tc.tile_pool(name="wt1", bufs=1) as wt1,
            tc.tile_pool(name="wg", bufs=1) as wg,
            tc.tile_pool(name="stream", bufs=3) as stream,
            tc.tile_pool(name="act1", bufs=1) as act1,
            tc.tile_pool(name="act2", bufs=2) as act2,
            tc.tile_pool(name="state", bufs=1) as state,
            tc.tile_pool(name="tmp", bufs=2) as tmp,
            tc.tile_pool(name="sm", bufs=5) as sm,
            tc.tile_pool(name="psA", bufs=2, space="PSUM") as psA,
            tc.tile_pool(name="psS", bufs=1, space="PSUM") as psS,
            tc.tile_pool(name="psM", bufs=2, space="PSUM") as psM,
            tc.tile_pool(name="psY", bufs=3, space="PSUM") as psY,
            tc.tile_pool(name="dram", bufs=2, space="DRAM") as dram,
        ):
            cx = Ctx()
            cx.wt, cx.wt1, cx.wg = wt, wt1, wg
            cx.stream, cx.act1, cx.act2 = stream, act1, act2
            cx.state, cx.tmp, cx.sm = state, tmp, sm
            cx.psA, cx.psS, cx.psM = psA, psS, psM
            cx.psY, cx.dram = psY, dram

            cx.onesB = cst.tile([128, 128], BF16, name="onesB")
            nc.vector.memset(cx.onesB[:], 1.0)
            cx.epsc = cst.tile([128, 1], F32, name="epsc")
            nc.vector.memset(cx.epsc[:], EPS)
            cx.spat = cst.tile([128, KT, TOK], F32, name="spatc")
            nc.sync.dma_start(cx.spat[:], spat.ap())
            cx.tpos = cst.tile([128, KT, L_RUN], F32, name="tposc")
            nc.sync.dma_start(cx.tpos[:], tpos.ap())
            cx.h0_in = h0_in

            x1_sc = dram.tile([L_RUN, 128, KT, TOK], F32, name="x1_sc", tag="x1_sc")

            # layer 1 runs (bwd, fwd): layer 0's SECOND scan (bwd) finishes
            # frame 11 first, which is exactly the frame layer-1-bwd needs
            # first -> the layer boundary pipelines instead of serializing.
            scan_list = []
            for layer in range(LAYERS_RUN):
                dirs = DIRS_RUN if layer == 0 else tuple(reversed(DIRS_RUN))
                for scan_i, d in enumerate(dirs):
                    scan_list.append((layer, scan_i, d))
            w_cur = None
            for order, (layer, scan_i, d) in enumerate(scan_list):
                si = layer * len(DIRS_RUN) + DIRS_RUN.index(d)
                if order == 0:
                    w_cur = _load_wqk8(nc, cx, segs[si], nc.sync)
                    w_cur.update(_load_seg_rest(nc, cx, segs[si], nc.sync))
                if scan_i == 0:
                    yf_sc = dram.tile([L_RUN, 128, KT, TOK], F32,
                                      name="yf_sc", tag="yf_sc")
                x_src = x_in.ap() if layer == 0 else x1_sc
                last_layer = layer == LAYERS_RUN - 1
                fwd = d == 0
                combine = scan_i == 1
                frames = (list(range(L_RUN)) if fwd
                          else list(range(L_RUN - 1, -1, -1)))
                if not combine:
                    out_dst = yf_sc
                elif last_layer:
                    out_dst = out_x.ap()
                else:
                    out_dst = x1_sc
                nsi = None
                if order + 1 < len(scan_list):
                    nl, _, nd = scan_list[order + 1]
                    nsi = nl * len(DIRS_RUN) + DIRS_RUN.index(nd)
                w_next = _emit_scan(nc, cx, segs[si], w_cur,
                                    segs[nsi] if nsi is not None else None,
                                    x_src, frames,
                                    pos_fixed=(layer if fwd else None),
                                    yf_sc=yf_sc, combine=combine,
                                    out_dst=out_dst)
                if nsi is not None:
                    # single-buffered tiles must load AFTER this scan's
                    # last emitted reader (emission order = data order)
                    w_next.update(_load_seg_rest(nc, cx, segs[nsi],
                                                 nc.gpsimd))
                    w_cur = w_next
    nc.compile()
    return nc


def _load_wqk8(nc, cx, seg, eng):
    """wqk8 is double-buffered so it can prefetch mid-scan (the k MMs are
    the first PE work of a scan)."""
    t = cx.wt.tile([128, 2, KT, 2 * D], F8, name="wqk8", tag="wqk8")
    eng.dma_start(t[:], seg["wqk8"].ap())
    return {"wqk8": t}


def _load_seg_rest(nc, cx, seg, eng):
    """Single-buffered weights: must be emitted after the previous
    segment's last reader (Tile orders by emission), i.e. at scan
    boundaries; all are needed >=7us into the scan so the DMA hides."""
    w = {}
    for nm, shape, dt, pool in (
            ("wv16", [128, 2, KT, D], F16, cx.wt1),
            ("B3qk", [65, 2 * D], F16, cx.wt1),
            ("B3v", [65, D], F16, cx.wt1),
            ("B2", [65, M], F16, cx.wt1),
            ("bout", [128, KT], F32, cx.wt1),
            ("b2", [128, KT], F32, cx.wt1),
            ("g1", [128, MT, KT, 128], F8, cx.wg)):
        w[nm] = pool.tile(shape, dt, name=nm, tag=nm)
        eng.dma_start(w[nm][:], seg[nm].ap())
    return w


def _emit_scan(nc, cx, seg, w, seg_next, x_src, frames, pos_fixed, yf_sc,
               combine, out_dst):
    # heff = h0 + pos[tp0] (f32 carry, maintained by the attention tail);
    # h0 borrows the yfld slot (idle at scan starts)
    h0t = cx.act1.tile([128, KT, TOK], F32, name="yfld", tag="yfld")
    nc.sync.dma_start(h0t[:], cx.h0_in.ap())
    # heff/bd16 rotate 2 buffers so the next scan's head (x-stage, h-LN,
    # k/v, its first all-reduce) overlaps this scan's tail frames
    heff = cx.act2.tile([128, KT, TOK], F32, name="heff", tag="heff")
    cx.bd16 = cx.wt1.tile([128, KT, 128], F16, name="bd16", tag="bd16")
    nc.vector.memset(cx.bd16[:], 0.0)  # off-diag blocks must stay zero
    tp0 = pos_fixed if pos_fixed is not None else frames[0]
    for kd in range(KT):
        nc.vector.scalar_tensor_tensor(
            heff[:, kd, :], cx.spat[:, kd, :], cx.tpos[:, kd, tp0:tp0 + 1],
            h0t[:, kd, :], op0=ALU.mult, op1=ALU.add)

    xs = _x_stage(nc, cx, x_src, frames[0])
    pend = None
    w_next = None
    for i, t in enumerate(frames):
        nxt = frames[i + 1] if i + 1 < len(frames) else None
        pend, xs = _emit_frame(nc, cx, seg, w, t, nxt, x_src, heff, xs,
                               pos_fixed, yf_sc, combine, out_dst, pend)
        if i == 1 and seg_next is not None:
            w_next = _load_wqk8(nc, cx, seg_next, nc.gpsimd)
    _emit_mlp(nc, cx, seg, w, pend)
    return w_next


def _x_stage(nc, cx, x_src, t):
    """x-side work for frame t: load, add pos, LN stats, normalize.

    Returns dict(xeff, zz, z16x, R3); zz plane 0 (x-side fp8) and z16x
    (f16 for the v matmuls) are written here; zz plane 1 / z16h by the
    h-stage. R3 rows [ones; mrb_x; <mrb_h>] (row 64 filled by h-stage).
    """
    # xeff doubles as x2 later (attn tail adds in place); lives until the
    # deferred MLP tail of this frame -> 3 buffers (stream pool)
    xeff = cx.stream.tile([128, KT, TOK], F32, name="xe", tag="xe")
    nc.sync.dma_start(xeff[:], x_src[t])
    for kd in range(KT):
        nc.vector.scalar_tensor_tensor(
            xeff[:, kd, :], cx.spat[:, kd, :], cx.tpos[:, kd, t:t + 1],
            xeff[:, kd, :], op0=ALU.mult, op1=ALU.add)
    xb, rbb, mean, rb32 = _ln_chain(nc, cx, xeff, "x")
    R3 = cx.act2.tile([65, TOK], F16, name="R3", tag="R3")
    nc.vector.memset(R3[:], 0.0)     # garbage rows x zero weights else NaN
    nc.vector.memset(R3[0:1, :], 1.0)
    # stats are partition-replicated; read partition 32 to write row 32
    nc.vector.tensor_mul(R3[32:33, :], mean[32:33, :], rb32[32:33, :])
    zz = cx.act2.tile([128, 2, KT, TOK], F8, name="zz", tag="zz")
    z16x = cx.act2.tile([128, KT, TOK], F16, name="z16x", tag="z16x")
    for kd in range(KT):
        nc.vector.tensor_mul(zz[:, 0, kd, :], xb[:, kd, 0:TOK], rbb[:])
        nc.vector.tensor_mul(z16x[:, kd, :], xb[:, kd, 0:TOK], rbb[:])
    return dict(xeff=xeff, zz=zz, z16x=z16x, R3=R3)


def _emit_frame(nc, cx, seg, w, t, nxt, x_src, heff, xs, pos_fixed, yf_sc,
                combine, out_dst, pend):
    tpn = pos_fixed if pos_fixed is not None else nxt   # next frame's pos idx

    xeff, zz, z16x, R3 = xs["xeff"], xs["zz"], xs["z16x"], xs["R3"]

    # ---- h-side LN (critical chain)
    hb, rbh, meanh, rb32h = _ln_chain(nc, cx, heff, "h")
    nc.vector.tensor_mul(R3[64:65, :], meanh[64:65, :], rb32h[64:65, :])
    z16h = cx.act1.tile([128, KT, TOK], F16, name="z16h", tag="z16h")
    for kd in range(KT):
        nc.vector.tensor_mul(zz[:, 1, kd, :], hb[:, kd, 0:TOK], rbh[:])
        nc.vector.tensor_mul(z16h[:, kd, :], hb[:, kd, 0:TOK], rbh[:])

    # ---- k (fp8 DR, cols [D,2D) of qkv = [768,1536) of wqk8) and
    #      v (f16, wv16) -- token-major: (128, 2, D) each [tok-half, feature]
    k16 = cx.act1.tile([128, 2, D], F16, name="k16", tag="k16")
    v16 = cx.act1.tile([128, 2, D], F16, name="v16", tag="v16")
    for tok2 in range(2):
        tsl = slice(tok2 * 128, (tok2 + 1) * 128)
        for ci, (lo, nc_) in enumerate(((D, 512), (D + 512, 256))):
            ps = cx.ps()
            for kd in range(KT):
                nc.tensor.matmul(ps[:, 0:nc_], zz[:, :, kd, tsl],
                                 w["wqk8"][:, :, kd, lo:lo + nc_],
                                 start=(kd == 0), stop=False,
                                 perf_mode=mybir.MatmulPerfMode.DoubleRow)
            nc.tensor.matmul(ps[:, 0:nc_], R3[:, tsl],
                             w["B3qk"][:, lo:lo + nc_], start=False, stop=True)
            off = lo - D
            _elu1(nc, cx, ps[:, 0:nc_], k16[:, tok2, off:off + nc_], nc_)
        for ci, (lo, nc_) in enumerate(((0, 512), (512, 256))):
            ps = cx.ps()
            for kd in range(KT):
                nc.tensor.matmul(ps[:, 0:nc_], z16x[:, kd, tsl],
                                 w["wv16"][:, 0, kd, lo:lo + nc_],
                                 start=(kd == 0), stop=False)
            for kd in range(KT):
                nc.tensor.matmul(ps[:, 0:nc_], z16h[:, kd, tsl],
                                 w["wv16"][:, 1, kd, lo:lo + nc_],
                                 start=False, stop=False)
            nc.tensor.matmul(ps[:, 0:nc_], R3[:, tsl],
                             w["B3v"][:, lo:lo + nc_], start=False, stop=True)
            nc.vector.tensor_copy(v16[:, tok2, lo:lo + nc_], ps[:, 0:nc_])

    # ---- kv state per head-pair; pack diag blocks into (128, 384) f32
    kvpack = cx.act1.tile([128, H * 32], F32, name="kvpack", tag="kvpack")
    for hp in range(KT):
        ps = cx.ps()
        pskv = ps[:, 0:128]
        for tok2 in range(2):
            nc.tensor.matmul(pskv, k16[:, tok2, hp * 128:(hp + 1) * 128],
                             v16[:, tok2, hp * 128:(hp + 1) * 128],
                             start=(tok2 == 0), stop=(tok2 == 1))
        # KVS fold happens here (pre-AR) so the post-AR bd16 is a plain cast
        nc.vector.tensor_scalar_mul(kvpack[0:64, hp * 64:(hp + 1) * 64],
                                    pskv[0:64, 0:64], KVS)
        nc.vector.tensor_scalar_mul(kvpack[64:128, hp * 64:(hp + 1) * 64],
                                    pskv[64:128, 64:128], KVS)

    # ---- all-reduce kv within the token-shard group
    arin = cx.dram.tile([128, H * 32], F32, name="arin", tag="arin")
    arout = cx.dram.tile([128, H * 32], F32, name="arout", tag="arout")
    nc.sync.dma_start(arin[:], kvpack[:])
    nc.gpsimd.collective_compute(
        "AllReduce", ALU.add, replica_groups=REPLICA_GROUPS,
        ins=[arin.opt()], outs=[arout.opt()])
    # kvred trigger issued NOW so it doesn't queue behind the MLP's
    # weight-stream triggers (sync queue is in-order)
    kvred = cx.act1.tile([128, H * 32], F32, name="kvred", tag="kvred")
    nc.sync.dma_start(kvred[:], arout[:])

    # ---- q (feature-major, fp8 DR over cols [0,768)): only needed after
    # kvred, so emitted after the collective launch -- its matmuls fill
    # the all-reduce window with real work
    q16 = cx.act1.tile([128, KT, TOK], F16, name="q16", tag="q16")
    for ft in range(KT):
        ps = cx.ps()
        for kd in range(KT):
            nc.tensor.matmul(ps[:, 0:TOK],
                             w["wqk8"][:, :, kd, ft * 128:(ft + 1) * 128],
                             zz[:, :, kd, :], start=(kd == 0), stop=False,
                             perf_mode=mybir.MatmulPerfMode.DoubleRow)
        nc.tensor.matmul(ps[:, 0:TOK], w["B3qk"][:, ft * 128:(ft + 1) * 128],
                         R3[:], start=False, stop=True)
        _elu1(nc, cx, ps[:, 0:TOK], q16[:, ft, :], TOK)

    # ---- prefetch wout ft-chunks for the attention GEMM
    wos = []
    for ft in range(KT):
        c = cx.stream.tile([128, KT, 128], F16, name="wos", tag="wos")
        nc.sync.dma_start(c[:], seg["wout"].ap()[ft])
        wos.append(c)

    # ---- next frame's x-side: its DVE chain fills the all-reduce window
    xs_next = _x_stage(nc, cx, x_src, nxt) if nxt is not None else None

    # ---- attention block: emitted BEFORE the deferred MLP so it has
    # higher list-scheduler priority and preempts leftover MLP work the
    # moment kvred lands; the MLP (always-ready, lower priority) fills
    # the all-reduce window and h-chain stalls.
    # block-diag kv (f16; KVS already folded pre-AR, wout carries the 256x)
    for hp in range(KT):
        nc.vector.tensor_copy(cx.bd16[0:64, hp, 0:64],
                              kvred[0:64, hp * 64:(hp + 1) * 64])
        nc.vector.tensor_copy(cx.bd16[64:128, hp, 64:128],
                              kvred[64:128, hp * 64:(hp + 1) * 64])
    o16 = cx.act1.tile([128, KT, TOK], F16, name="o16", tag="o16")
    for hp in range(KT):
        ps = cx.ps()
        nc.tensor.matmul(ps[:, 0:TOK], cx.bd16[:, hp, :], q16[:, hp, :],
                         start=True, stop=True)
        nc.vector.tensor_copy(o16[:, hp, :], ps[:, 0:TOK])

    # attn (feature-major); (attn+bout) gathered into at32 on DVE,
    # then two wide DVE adds + per-ft pos STT update x2 and heff
    at32 = cx.act1.tile([128, KT, TOK], F32, name="at32", tag="at32")
    for ft in range(KT):
        ps = cx.ps()
        for hp in range(KT):
            nc.tensor.matmul(ps[:, 0:TOK], wos[ft][:, hp, :],
                             o16[:, hp, :], start=(hp == 0), stop=(hp == KT - 1))
        nc.vector.tensor_scalar_add(at32[:, ft, :], ps[:, 0:TOK],
                                    w["bout"][:, ft:ft + 1])
    # x2 = attn + x_eff, in place over xeff (must read at32 before the
    # heff update below overwrites it)
    nc.vector.tensor_add(xeff[:], at32[:], xeff[:])
    if nxt is not None:
        nc.vector.tensor_add(at32[:], at32[:], heff[:])
        for ft in range(KT):
            nc.vector.scalar_tensor_tensor(
                heff[:, ft, :], cx.spat[:, ft, :], cx.tpos[:, ft, tpn:tpn + 1],
                at32[:, ft, :], op0=ALU.mult, op1=ALU.add)

    # ---- output LN -> z2 for the deferred MLP; emitted right after the
    # attention tail so MLP(t) (which fills frame t+1's h-chain stall)
    # becomes ready as early as possible
    # o-chain borrows the h-chain's xq buffer: h-LN(t)'s xq readers are
    # done once zz/z16h are written, well before the attention tail
    ob, rbo, meano, rb32o = _ln_chain(nc, cx, xeff, "o", buf_tag="xb_h")
    R2 = cx.act2.tile([65, TOK], F16, name="R2", tag="R2")
    nc.vector.memset(R2[:], 0.0)     # garbage rows x zero weights else NaN
    nc.vector.memset(R2[0:1, :], 1.0)
    nc.vector.tensor_mul(R2[32:33, :], meano[32:33, :], rb32o[32:33, :])
    z2 = cx.act2.tile([128, KT, TOK], F8, name="z2", tag="z2")
    for kd in range(KT):
        nc.vector.tensor_mul(z2[:, kd, :], ob[:, kd, 0:TOK], rbo[:])

    # ---- deferred MLP of the previous frame (hides the all-reduce)
    if pend is not None:
        _emit_mlp(nc, cx, seg, w, pend)

    pend = dict(t=t, z2=z2, R2=R2, x232=xeff, combine=combine,
                out_dst=out_dst, yf_sc=yf_sc)
    return pend, xs_next


def _emit_mlp(nc, cx, seg, w, pend):
    t, z2, R2, x232 = pend["t"], pend["z2"], pend["R2"], pend["x232"]
    combine, out_dst, yf_sc = pend["combine"], pend["out_dst"], pend["yf_sc"]

    # y2 accumulators pair two ft per PSUM bank (3 banks total)
    yps = [cx.psY.tile([128, 2 * TOK], F32, name="psy", tag="psy")
           for _ in range(KT // 2)]

    def ypsl(ft):
        return yps[ft // 2][:, (ft % 2) * TOK:(ft % 2 + 1) * TOK]

    DR = mybir.MatmulPerfMode.DoubleRow
    for mjp in range(MT // 2):
        # bulk weight streams ride the gpsimd SW-DGE queue so their
        # slot-waits never block the sync queue's latency DMAs
        w2s = cx.stream.tile([128, 2, D], F8, name="w2s", tag="w2s")
        nc.gpsimd.dma_start(w2s[:], seg["w2p"].ap()[mjp])
        y1p = cx.stream.tile([128, 2, TOK], F8, name="y1p", tag="y1p")
        # both mj halves share one psM bank -> a single 512-wide gelu
        # (ACT_TABLE_LOAD is 1283ns; halving gelu count halves the thrash)
        ps = cx.psM.tile([128, 2 * TOK], F32, name="psm", tag="psm")
        for i in range(2):
            mj = 2 * mjp + i
            sl = slice(i * TOK, (i + 1) * TOK)
            for kdp in range(KT // 2):
                nc.tensor.matmul(ps[:, sl],
                                 w["g1"][:, mj, 2 * kdp:2 * kdp + 2, :],
                                 z2[:, 2 * kdp:2 * kdp + 2, :],
                                 start=(kdp == 0), stop=False, perf_mode=DR)
            nc.tensor.matmul(ps[:, sl], w["B2"][:, mj * 128:(mj + 1) * 128],
                             R2[:], start=False, stop=True)
        nc.scalar.activation(y1p[:, :, :], ps[:], AF.Gelu, scale=W8SI)
        for ft in range(KT):
            nc.tensor.matmul(ypsl(ft), w2s[:, :, ft * 128:(ft + 1) * 128],
                             y1p[:], start=(mjp == 0), stop=(mjp == MT // 2 - 1),
                             perf_mode=DR)

    if not combine:
        for ft in range(KT):
            yb = cx.tmp.tile([128, TOK], F32, name="yb", tag="yb")
            nc.vector.tensor_scalar(yb[:], ypsl(ft), W8SI,
                                    w["b2"][:, ft:ft + 1],
                                    op0=ALU.mult, op1=ALU.add)
            nc.vector.tensor_add(x232[:, ft, :], yb[:], x232[:, ft, :])
    else:
        yf = cx.act1.tile([128, KT, TOK], F32, name="yfld", tag="yfld")
        nc.sync.dma_start(yf[:], yf_sc[t])
        for ft in range(KT):
            yb = cx.tmp.tile([128, TOK], F32, name="yb", tag="yb")
            nc.vector.tensor_scalar(yb[:], ypsl(ft), W8SI,
                                    w["b2"][:, ft:ft + 1],
                                    op0=ALU.mult, op1=ALU.add)
            nc.vector.tensor_add(yb[:], yb[:], yf[:, ft, :])
            nc.vector.tensor_add(x232[:, ft, :], yb[:], x232[:, ft, :])
    nc.sync.dma_start(out_dst[t], x232[:])


# ---------------------------------------------------------------- entry point

@functools.cache
def _compiled_nc():
    return build_nc()


def kernel(**inputs):
    inputs = {k: np.asarray(v) for k, v in inputs.items()}
    nc = _compiled_nc()
    in_maps = make_in_maps(inputs)
    res = run_bass_kernel_spmd(nc, in_maps, list(range(NCORES)))
    return unshard_output(res.results)


# revision 43
# speedup vs baseline: 1.1281x; 1.1281x over previous
"""Trainium2 Bass kernel for nn_GPTrack2D (dense transformer with linear
attention and a per-frame recurrence over L).

Sharding: batch (2) -> two groups of 4 cores; tokens (1024 -> 256/core)
within each group. Linear attention's k^T v state is all-reduced per frame
within the group; the all-reduce hides behind the previous frame's MLP and
the next frame's x-side LayerNorm (software-pipelined emission).

v3 over v2: fp8(e4m3) DoubleRow matmuls where numerics allow.
- q,k projections: fp8 DR pairing the x-side and h-side contractions
  (zz tile holds fp8 zx/zh planes; wqk8 holds both weight planes, x64).
- v projection stays f16 (fp8 v decorrelates the recurrence: sim showed
  6.8% rel err with v fp8 vs 0.57% with v f16).
- MLP: both GEMMs fp8 DR (g1 pairs kd, w2 pairs mj); numerically free.
- elu+1 on a x64-scaled psum via  max(x,0) + min(exp(x),1)  keeps the
  3-op structure with the 1/64 descale folded into ScalarE's scale arg.
- B2 bias rows padded 33->65 (rank-33 matmuls measured 237ns vs 134).
- Identity copies moved from ScalarE to DVE (ACT_TABLE_LOAD thrash:
  1283ns per function switch, was ~21/frame).
- wqk8/wv16 double-buffered; next segment prefetched mid-scan on the
  gpsimd queue, so scan boundaries don't stall on weight DMA.
- g1 fully SBUF-resident in fp8 (no streamed half).

Precision: residual stream / carry f32; LN stat inputs bf16 (|h| reaches
~1.3e6). kv state scaled by 1/256 to fit f16; wout pre-scaled by 256.
fp8 weights pre-scaled by 64 (0.02-scale weights underflow e4m3 norms).
"""

import functools

import numpy as np
import ml_dtypes

import concourse.bacc as bacc
import concourse.mybir as mybir
from concourse import tile
from concourse.bass_utils import run_bass_kernel_spmd

F32 = mybir.dt.float32
BF16 = mybir.dt.bfloat16
F16 = mybir.dt.float16
F8 = mybir.dt.float8e4
NP_F8 = ml_dtypes.float8_e4m3
AF = mybir.ActivationFunctionType
ALU = mybir.AluOpType

B, L, N, D, M, H = 2, 12, 1024, 768, 3072, 12
NCORES = 8
GROUP = 4                 # cores per batch group
TOK = N // GROUP          # 256 tokens per core
KT = D // 128             # 6 d-tiles
MT = M // 128             # 24 m-tiles
F3 = 3 * D                # 2304
EPS = 1e-5
KVS = 1.0 / 256.0         # kv-state scale so fp16 holds it
KVSI = 256.0
W8S = 64.0                # fp8 weight pre-scale
W8SI = 1.0 / 64.0

# dev-scale knobs (full problem: L_RUN=12, LAYERS_RUN=2, DIRS_RUN=(0, 1))
L_RUN = L
LAYERS_RUN = 2
DIRS_RUN = (0, 1)

REPLICA_GROUPS = [[0, 1, 2, 3], [4, 5, 6, 7]]


# ---------------------------------------------------------------- host prep

def _pack_weights(inputs):
    segs = []
    for layer in range(LAYERS_RUN):
        for d in DIRS_RUN:
            gi = np.asarray(inputs["lni_g"][d, layer]); bi = np.asarray(inputs["lni_b"][d, layer])
            gh = np.asarray(inputs["lnh_g"][d, layer]); bh = np.asarray(inputs["lnh_b"][d, layer])
            go = np.asarray(inputs["lno_g"][d, layer]); bo = np.asarray(inputs["lno_b"][d, layer])
            Wqkv = np.asarray(inputs["Wqkv"][d, layer]); bqkv = np.asarray(inputs["bqkv"][d, layer])
            Wqkvh = np.asarray(inputs["Wqkvh"][d, layer]); bqkvh = np.asarray(inputs["bqkvh"][d, layer])
            Wout = np.asarray(inputs["Wout"][d, layer]); bout = np.asarray(inputs["bout"][d, layer])
            W1 = np.asarray(inputs["W1"][d, layer]); b1 = np.asarray(inputs["b1"][d, layer])
            W2 = np.asarray(inputs["W2"][d, layer]); b2 = np.asarray(inputs["b2"][d, layer])

            gqkv = gi[:, None] * Wqkv                      # (D, 3D)
            gqkvh = gh[:, None] * Wqkvh
            cqkv = bi @ Wqkv + bqkv + bh @ Wqkvh + bqkvh   # (3D,)
            # [p, side, kd, f]: side 0 = x-weights, 1 = h-weights
            wpair = np.stack([
                gqkv.reshape(KT, 128, F3).transpose(1, 0, 2),
                gqkvh.reshape(KT, 128, F3).transpose(1, 0, 2)], axis=1)
            # q,k columns [0, 2D) in fp8 x64; v columns [2D, 3D) in f16
            wqk8 = np.ascontiguousarray(wpair[:, :, :, :2 * D] * W8S).astype(NP_F8)
            wv16 = np.ascontiguousarray(wpair[:, :, :, 2 * D:]).astype(np.float16)
            # bias rows on partitions 0/32/64 (DVE base-partition rule)
            B3 = np.zeros((65, F3), np.float32)
            B3[0], B3[32], B3[64] = cqkv, -gqkv.sum(0), -gqkvh.sum(0)
            B3qk = (B3[:, :2 * D] * W8S).astype(np.float16)
            B3v = B3[:, 2 * D:].astype(np.float16)

            g1 = go[:, None] * W1                          # (D, M)
            c1 = bo @ W1 + b1                              # (M,)
            B2 = np.zeros((65, M), np.float32)
            B2[0], B2[32] = c1 * W8S, -g1.sum(0) * W8S

            seg = dict(
                wqk8=wqk8, wv16=wv16, B3qk=B3qk, B3v=B3v,
                # wout pre-scaled by KVSI: cancels the f16 kv-state 1/256.
                # ft-chunked for streaming: [ft, p, kd, c] = w[kd*128+p, ft*128+c]
                wout=np.ascontiguousarray(
                    (Wout * KVSI).reshape(KT, 128, KT, 128)
                    .transpose(2, 1, 0, 3)).astype(np.float16),
                bout=np.ascontiguousarray(
                    bout.reshape(KT, 128).T).astype(np.float32),
                # fully resident: [p, mj, kd, c] = g1[kd*128+p, mj*128+c]
                g1=np.ascontiguousarray(
                    (g1 * W8S).reshape(KT, 128, MT, 128)
                    .transpose(1, 2, 0, 3)).astype(NP_F8),
                B2=B2.astype(np.float16),
                # mj-paired for DR: [mjp, p, i, f] = W2[(2mjp+i)*128+p, f]
                w2p=np.ascontiguousarray(
                    (W2 * W8S).reshape(MT // 2, 2, 128, D)
                    .transpose(0, 2, 1, 3)).astype(NP_F8),
                b2=np.ascontiguousarray(
                    b2.reshape(KT, 128).T).astype(np.float32),    # (128, KT)
            )
            segs.append(seg)
    return segs


def _feat_major(a, dtype):
    """(..., tok, D) -> (..., 128, KT, tok) tiled feature-major."""
    t = np.moveaxis(np.asarray(a), -1, -2)                # (..., D, tok)
    shp = t.shape[:-2]
    t = t.reshape(shp + (KT, 128, t.shape[-1]))           # (..., KT, 128, tok)
    t = np.moveaxis(t, -3, -2)                            # (..., 128, KT, tok)
    return np.ascontiguousarray(t).astype(dtype)


def make_in_maps(inputs):
    segs = _pack_weights(inputs)
    in_maps = []
    for core in range(NCORES):
        b = core // GROUP
        s = (core % GROUP) * TOK
        m = {}
        m["x_in"] = _feat_major(
            np.asarray(inputs["x"])[b, :L_RUN, s:s + TOK, :], np.float32)
        m["h0_in"] = _feat_major(
            np.asarray(inputs["hidden"])[b, s:s + TOK, :], np.float32)
        m["spat"] = _feat_major(
            np.asarray(inputs["spatial_pos"])[b, s:s + TOK, :], np.float32)
        tp = np.asarray(inputs["temporal_pos"])[b, :L_RUN, :]   # (L, D)
        tp = tp.T.reshape(KT, 128, L_RUN).transpose(1, 0, 2)
        m["tpos"] = np.ascontiguousarray(tp).astype(np.float32)  # (128, KT, L)
        for si, seg in enumerate(segs):
            for k, v in seg.items():
                m[f"{k}_{si}"] = v
        in_maps.append(m)
    return in_maps


def unshard_output(results):
    out = np.empty((B, L_RUN, N, D), np.float32)
    for core in range(NCORES):
        b = core // GROUP
        s = (core % GROUP) * TOK
        o = np.asarray(results[core]["out_x"])            # (L, 128, KT, TOK)
        o = o.transpose(0, 2, 1, 3).reshape(L_RUN, D, TOK)
        out[b, :, s:s + TOK, :] = np.moveaxis(o, -1, -2)
    return out


# ---------------------------------------------------------------- kernel build

class Ctx:
    """Pools, constants and persistent tiles used during emission."""

    def ps(self):
        return self.psA.tile([128, 2 * TOK], F32, name="ps", tag="ps")


def _ln_chain(nc, cx, src32, tag, veng=None, buf_tag=None):
    """Feature-major LN for an SBUF (128, KT, TOK) f32 tile.

    Emits: bf16 copy xb, squares, packed stats matmuls (s1|s2 in one
    PSUM bank), mean/var smalls, sqrt + fast reciprocal. Returns (xb,
    rbb, mean, rb32). veng places the wide copy/square ops (gpsimd for
    off-critical chains keeps the DVE queue short).
    """
    veng = veng or nc.vector
    # xq packs [bf16 copy | its square] so one 512-wide matmul per kd
    # yields both stat sums; squares off ScalarE keep its LUT unthrashed
    xq = cx.act1.tile([128, KT, 2 * TOK], BF16, name=f"xb_{tag}",
                      tag=buf_tag or f"xb_{tag}")
    s12 = cx.psS.tile([128, 2 * TOK], F32, name="s12", tag="s12")
    for kd in range(KT):
        veng.tensor_copy(xq[:, kd, 0:TOK], src32[:, kd, :])
        veng.tensor_mul(xq[:, kd, TOK:2 * TOK], xq[:, kd, 0:TOK],
                        xq[:, kd, 0:TOK])
        nc.tensor.matmul(s12[:], cx.onesB[:], xq[:, kd, :],
                         start=(kd == 0), stop=(kd == KT - 1))
    mean = cx.sm.tile([128, TOK], F32, name="mean", tag="lnsm")
    nc.vector.tensor_scalar_mul(mean[:], s12[:, 0:TOK], 1.0 / D)
    msq = cx.sm.tile([128, TOK], F32, name="msq", tag="lnsm")
    nc.vector.tensor_mul(msq[:], mean[:], mean[:])
    ve = cx.sm.tile([128, TOK], F32, name="ve", tag="lnsm")
    nc.vector.scalar_tensor_tensor(ve[:], s12[:, TOK:2 * TOK], 1.0 / D, msq[:],
                                   op0=ALU.mult, op1=ALU.subtract)
    rb32 = cx.sm.tile([128, TOK], F32, name="rb32", tag="lnsm")
    # 1/sqrt(|ve|+eps) in one ScalarE op (Rsqrt is banned; this isn't)
    nc.scalar.activation(rb32[:], ve[:], AF.Abs_reciprocal_sqrt,
                         bias=cx.epsc[:])
    rbb = cx.tmp.tile([128, TOK], BF16, name=f"rbb_{tag}", tag=f"rbb_{tag}")
    nc.vector.tensor_copy(rbb[:], rb32[:])
    return xq, rbb, mean, rb32


def _elu1(nc, cx, psum_ap, out_ap, ncols):
    """out = elu(x)+1 where psum holds 64*x.

    max(x,0) + min(exp(x),1): tlin and texp both read the psum directly
    (no serial DVE->ScalarE chain); ScalarE folds the 1/64 descale.
    """
    tlin = cx.act1.tile([128, 512], F32, name="elin", tag="elin")
    texp = cx.act1.tile([128, 512], F32, name="eexp", tag="eexp")
    nc.vector.tensor_scalar(tlin[:, :ncols], psum_ap, W8SI, 0.0,
                            op0=ALU.mult, op1=ALU.max)
    nc.scalar.activation(texp[:, :ncols], psum_ap, AF.Exp, scale=W8SI)
    nc.vector.scalar_tensor_tensor(out_ap, texp[:, :ncols], 1.0,
                                   tlin[:, :ncols], op0=ALU.min, op1=ALU.add)


def build_nc():
    nc = bacc.Bacc("TRN2", target_bir_lowering=False, debug=False,
                   num_devices=NCORES)

    x_in = nc.dram_tensor("x_in", [L_RUN, 128, KT, TOK], F32, kind="ExternalInput")
    h0_in = nc.dram_tensor("h0_in", [128, KT, TOK], F32, kind="ExternalInput")
    spat = nc.dram_tensor("spat", [128, KT, TOK], F32, kind="ExternalInput")
    tpos = nc.dram_tensor("tpos", [128, KT, L_RUN], F32, kind="ExternalInput")
    nseg = LAYERS_RUN * len(DIRS_RUN)
    segs = []
    for si in range(nseg):
        segs.append(dict(
            wqk8=nc.dram_tensor(f"wqk8_{si}", [128, 2, KT, 2 * D], F8, kind="ExternalInput"),
            wv16=nc.dram_tensor(f"wv16_{si}", [128, 2, KT, D], F16, kind="ExternalInput"),
            B3qk=nc.dram_tensor(f"B3qk_{si}", [65, 2 * D], F16, kind="ExternalInput"),
            B3v=nc.dram_tensor(f"B3v_{si}", [65, D], F16, kind="ExternalInput"),
            wout=nc.dram_tensor(f"wout_{si}", [KT, 128, KT, 128], F16, kind="ExternalInput"),
            bout=nc.dram_tensor(f"bout_{si}", [128, KT], F32, kind="ExternalInput"),
            g1=nc.dram_tensor(f"g1_{si}", [128, MT, KT, 128], F8, kind="ExternalInput"),
            B2=nc.dram_tensor(f"B2_{si}", [65, M], F16, kind="ExternalInput"),
            w2p=nc.dram_tensor(f"w2p_{si}", [MT // 2, 128, 2, D], F8, kind="ExternalInput"),
            b2=nc.dram_tensor(f"b2_{si}", [128, KT], F32, kind="ExternalInput"),
        ))
    out_x = nc.dram_tensor("out_x", [L_RUN, 128, KT, TOK], F32, kind="ExternalOutput")

    with tile.TileContext(nc) as tc:
        with (
            tc.tile_pool(name="cst", bufs=1) as cst,
            tc.tile_pool(name="wt", bufs=2) as wt,
            tc.tile_pool(name="wt1", bufs=1) as wt1,
            tc.tile_pool(name="wg", bufs=1) as wg,
            tc.tile_pool(name="stream", bufs=3) as stream,
            tc.tile_pool(name="act1", bufs=1) as act1,
            tc.tile_pool(name="act2", bufs=2) as act2,
            tc.tile_pool(name="state", bufs=1) as state,
            tc.tile_pool(name="tmp", bufs=2) as tmp,
            tc.tile_pool(name="sm", bufs=5) as sm,
            tc.tile_pool(name="psA", bufs=2, space="PSUM") as psA,
            tc.tile_pool(name="psS", bufs=1, space="PSUM") as psS,
            tc.tile_pool(name="psM", bufs=2, space="PSUM") as psM,
            tc.tile_pool(name="psY", bufs=3, space="PSUM") as psY,
            tc.tile_pool(name="dram", bufs=2, space="DRAM") as dram,
        ):
            cx = Ctx()
            cx.wt, cx.wt1, cx.wg = wt, wt1, wg
            cx.stream, cx.act1, cx.act2 = stream, act1, act2
            cx.state, cx.tmp, cx.sm = state, tmp, sm
            cx.psA, cx.psS, cx.psM = psA, psS, psM
            cx.psY, cx.dram = psY, dram

            cx.onesB = cst.tile([128, 128], BF16, name="onesB")
            nc.vector.memset(cx.onesB[:], 1.0)
            cx.epsc = cst.tile([128, 1], F32, name="epsc")
            nc.vector.memset(cx.epsc[:], EPS)
            cx.spat = cst.tile([128, KT, TOK], F32, name="spatc")
            nc.sync.dma_start(cx.spat[:], spat.ap())
            cx.tpos = cst.tile([128, KT, L_RUN], F32, name="tposc")
            nc.sync.dma_start(cx.tpos[:], tpos.ap())
            cx.h0_in = h0_in

            x1_sc = dram.tile([L_RUN, 128, KT, TOK], F32, name="x1_sc", tag="x1_sc")

            # layer 1 runs (bwd, fwd): layer 0's SECOND scan (bwd) finishes
            # frame 11 first, which is exactly the frame layer-1-bwd needs
            # first -> the layer boundary pipelines instead of serializing.
            scan_list = []
            for layer in range(LAYERS_RUN):
                dirs = DIRS_RUN if layer == 0 else tuple(reversed(DIRS_RUN))
                for scan_i, d in enumerate(dirs):
                    scan_list.append((layer, scan_i, d))
            w_cur = None
            for order, (layer, scan_i, d) in enumerate(scan_list):
                si = layer * len(DIRS_RUN) + DIRS_RUN.index(d)
                if order == 0:
                    w_cur = _load_wqk8(nc, cx, segs[si], nc.sync)
                    w_cur.update(_load_seg_rest(nc, cx, segs[si], nc.sync))
                if scan_i == 0:
                    yf_sc = dram.tile([L_RUN, 128, KT, TOK], F32,
                                      name="yf_sc", tag="yf_sc")
                x_src = x_in.ap() if layer == 0 else x1_sc
                last_layer = layer == LAYERS_RUN - 1
                fwd = d == 0
                combine = scan_i == 1
                frames = (list(range(L_RUN)) if fwd
                          else list(range(L_RUN - 1, -1, -1)))
                if not combine:
                    out_dst = yf_sc
                elif last_layer:
                    out_dst = out_x.ap()
                else:
                    out_dst = x1_sc
                nsi = None
                if order + 1 < len(scan_list):
                    nl, _, nd = scan_list[order + 1]
                    nsi = nl * len(DIRS_RUN) + DIRS_RUN.index(nd)
                w_next = _emit_scan(nc, cx, segs[si], w_cur,
                                    segs[nsi] if nsi is not None else None,
                                    x_src, frames,
                                    pos_fixed=(layer if fwd else None),
                                    yf_sc=yf_sc, combine=combine,
                                    out_dst=out_dst)
                if nsi is not None:
                    # single-buffered tiles must load AFTER this scan's
                    # last emitted reader (emission order = data order)
                    w_next.update(_load_seg_rest(nc, cx, segs[nsi],
                                                 nc.gpsimd))
                    w_cur = w_next
    nc.compile()
    return nc


def _load_wqk8(nc, cx, seg, eng):
    """wqk8 is double-buffered so it can prefetch mid-scan (the k MMs are
    the first PE work of a scan)."""
    t = cx.wt.tile([128, 2, KT, 2 * D], F8, name="wqk8", tag="wqk8")
    eng.dma_start(t[:], seg["wqk8"].ap())
    return {"wqk8": t}


def _load_seg_rest(nc, cx, seg, eng):
    """Single-buffered weights: must be emitted after the previous
    segment's last reader (Tile orders by emission), i.e. at scan
    boundaries; all are needed >=7us into the scan so the DMA hides."""
    w = {}
    for nm, shape, dt, pool in (
            ("wv16", [128, 2, KT, D], F16, cx.wt1),
            ("B3qk", [65, 2 * D], F16, cx.wt1),
            ("B3v", [65, D], F16, cx.wt1),
            ("B2", [65, M], F16, cx.wt1),
            ("bout", [128, KT], F32, cx.wt1),
            ("b2", [128, KT], F32, cx.wt1),
            ("g1", [128, MT, KT, 128], F8, cx.wg)):
        w[nm] = pool.tile(shape, dt, name=nm, tag=nm)
        eng.dma_start(w[nm][:], seg[nm].ap())
    return w


def _emit_scan(nc, cx, seg, w, seg_next, x_src, frames, pos_fixed, yf_sc,
               combine, out_dst):
    # heff = h0 + pos[tp0] (f32 carry, maintained by the attention tail);
    # h0 borrows the yfld slot (idle at scan starts)
    h0t = cx.act1.tile([128, KT, TOK], F32, name="yfld", tag="yfld")
    nc.sync.dma_start(h0t[:], cx.h0_in.ap())
    # heff/bd16 rotate 2 buffers so the next scan's head (x-stage, h-LN,
    # k/v, its first all-reduce) overlaps this scan's tail frames
    heff = cx.act2.tile([128, KT, TOK], F32, name="heff", tag="heff")
    cx.bd16 = cx.wt1.tile([128, KT, 128], F16, name="bd16", tag="bd16")
    nc.vector.memset(cx.bd16[:], 0.0)  # off-diag blocks must stay zero
    tp0 = pos_fixed if pos_fixed is not None else frames[0]
    for kd in range(KT):
        nc.vector.scalar_tensor_tensor(
            heff[:, kd, :], cx.spat[:, kd, :], cx.tpos[:, kd, tp0:tp0 + 1],
            h0t[:, kd, :], op0=ALU.mult, op1=ALU.add)

    xs = _x_stage(nc, cx, x_src, frames[0])
    pend = None
    w_next = None
    for i, t in enumerate(frames):
        nxt = frames[i + 1] if i + 1 < len(frames) else None
        pend, xs = _emit_frame(nc, cx, seg, w, t, nxt, x_src, heff, xs,
                               pos_fixed, yf_sc, combine, out_dst, pend)
        if i == 1 and seg_next is not None:
            w_next = _load_wqk8(nc, cx, seg_next, nc.gpsimd)
    _emit_mlp(nc, cx, seg, w, pend)
    return w_next


def _x_stage(nc, cx, x_src, t):
    """x-side work for frame t: load, add pos, LN stats, normalize.

    Returns dict(xeff, zz, z16x, R3); zz plane 0 (x-side fp8) and z16x
    (f16 for the v matmuls) are written here; zz plane 1 / z16h by the
    h-stage. R3 rows [ones; mrb_x; <mrb_h>] (row 64 filled by h-stage).
    """
    # xeff doubles as x2 later (attn tail adds in place); lives until the
    # deferred MLP tail of this frame -> 3 buffers (stream pool)
    xeff = cx.stream.tile([128, KT, TOK], F32, name="xe", tag="xe")
    nc.sync.dma_start(xeff[:], x_src[t])
    for kd in range(KT):
        nc.vector.scalar_tensor_tensor(
            xeff[:, kd, :], cx.spat[:, kd, :], cx.tpos[:, kd, t:t + 1],
            xeff[:, kd, :], op0=ALU.mult, op1=ALU.add)
    xb, rbb, mean, rb32 = _ln_chain(nc, cx, xeff, "x")
    R3 = cx.act2.tile([65, TOK], F16, name="R3", tag="R3")
    nc.vector.memset(R3[:], 0.0)     # garbage rows x zero weights else NaN
    nc.vector.memset(R3[0:1, :], 1.0)
    # stats are partition-replicated; read partition 32 to write row 32
    nc.vector.tensor_mul(R3[32:33, :], mean[32:33, :], rb32[32:33, :])
    zz = cx.act2.tile([128, 2, KT, TOK], F8, name="zz", tag="zz")
    z16x = cx.act2.tile([128, KT, TOK], F16, name="z16x", tag="z16x")
    for kd in range(KT):
        nc.vector.tensor_mul(zz[:, 0, kd, :], xb[:, kd, 0:TOK], rbb[:])
        nc.vector.tensor_mul(z16x[:, kd, :], xb[:, kd, 0:TOK], rbb[:])
    return dict(xeff=xeff, zz=zz, z16x=z16x, R3=R3)


def _emit_frame(nc, cx, seg, w, t, nxt, x_src, heff, xs, pos_fixed, yf_sc,
                combine, out_dst, pend):
    tpn = pos_fixed if pos_fixed is not None else nxt   # next frame's pos idx

    xeff, zz, z16x, R3 = xs["xeff"], xs["zz"], xs["z16x"], xs["R3"]

    # ---- h-side LN (critical chain)
    hb, rbh, meanh, rb32h = _ln_chain(nc, cx, heff, "h")
    nc.vector.tensor_mul(R3[64:65, :], meanh[64:65, :], rb32h[64:65, :])
    z16h = cx.act1.tile([128, KT, TOK], F16, name="z16h", tag="z16h")
    for kd in range(KT):
        nc.vector.tensor_mul(zz[:, 1, kd, :], hb[:, kd, 0:TOK], rbh[:])
        nc.vector.tensor_mul(z16h[:, kd, :], hb[:, kd, 0:TOK], rbh[:])

    # ---- k (fp8 DR, cols [D,2D) of qkv = [768,1536) of wqk8) and
    #      v (f16, wv16) -- token-major: (128, 2, D) each [tok-half, feature]
    # 512-wide chunks get a full psum bank; the two 256-wide tails share
    # one bank so every bank carries ~2x MM work per elu/copy hold (a
    # 2-buffer rotation starves when the reader latency exceeds the MM
    # group time).
    k16 = cx.act1.tile([128, 2, D], F16, name="k16", tag="k16")
    v16 = cx.act1.tile([128, 2, D], F16, name="v16", tag="v16")

    def kmm(ps_ap, tok2, lo, nc_):
        tsl = slice(tok2 * 128, (tok2 + 1) * 128)
        for kd in range(KT):
            nc.tensor.matmul(ps_ap, zz[:, :, kd, tsl],
                             w["wqk8"][:, :, kd, lo:lo + nc_],
                             start=(kd == 0), stop=False,
                             perf_mode=mybir.MatmulPerfMode.DoubleRow)
        nc.tensor.matmul(ps_ap, R3[:, tsl],
                         w["B3qk"][:, lo:lo + nc_], start=False, stop=True)

    def vmm(ps_ap, tok2, lo, nc_):
        tsl = slice(tok2 * 128, (tok2 + 1) * 128)
        for kd in range(KT):
            nc.tensor.matmul(ps_ap, z16x[:, kd, tsl],
                             w["wv16"][:, 0, kd, lo:lo + nc_],
                             start=(kd == 0), stop=False)
        for kd in range(KT):
            nc.tensor.matmul(ps_ap, z16h[:, kd, tsl],
                             w["wv16"][:, 1, kd, lo:lo + nc_],
                             start=False, stop=False)
        nc.tensor.matmul(ps_ap, R3[:, tsl],
                         w["B3v"][:, lo:lo + nc_], start=False, stop=True)

    for tok2 in range(2):
        ps = cx.ps()
        kmm(ps[:, 0:512], tok2, D, 512)
        _elu1(nc, cx, ps[:, 0:512], k16[:, tok2, 0:512], 512)
    ps = cx.ps()
    for tok2 in range(2):
        kmm(ps[:, tok2 * TOK:(tok2 + 1) * TOK], tok2, D + 512, 256)
    _elu1(nc, cx, ps[:], k16[:, :, 512:768], 512)
    for tok2 in range(2):
        ps = cx.ps()
        vmm(ps[:, 0:512], tok2, 0, 512)
        nc.vector.tensor_copy(v16[:, tok2, 0:512], ps[:, 0:512])
    ps = cx.ps()
    for tok2 in range(2):
        vmm(ps[:, tok2 * TOK:(tok2 + 1) * TOK], tok2, 512, 256)
    nc.vector.tensor_copy(v16[:, :, 512:768], ps[:])

    # ---- kv state per head-pair; pack diag blocks into (128, 384) f32;
    # three head-pairs per psum bank, two strided reads per bank
    kvpack = cx.act1.tile([128, H * 32], F32, name="kvpack", tag="kvpack")
    for bk in range(2):
        ps = cx.ps()
        for j in range(3):
            hp = 3 * bk + j
            for tok2 in range(2):
                nc.tensor.matmul(ps[:, j * 128:j * 128 + 128],
                                 k16[:, tok2, hp * 128:(hp + 1) * 128],
                                 v16[:, tok2, hp * 128:(hp + 1) * 128],
                                 start=(tok2 == 0), stop=(tok2 == 1))
        # KVS fold happens here (pre-AR) so the post-AR bd16 is a plain cast
        for j in range(3):
            hp = 3 * bk + j
            nc.vector.tensor_scalar_mul(kvpack[0:64, hp * 64:(hp + 1) * 64],
                                        ps[0:64, j * 128:j * 128 + 64], KVS)
            nc.vector.tensor_scalar_mul(kvpack[64:128, hp * 64:(hp + 1) * 64],
                                        ps[64:128, j * 128 + 64:j * 128 + 128],
                                        KVS)

    # ---- all-reduce kv within the token-shard group
    arin = cx.dram.tile([128, H * 32], F32, name="arin", tag="arin")
    arout = cx.dram.tile([128, H * 32], F32, name="arout", tag="arout")
    nc.sync.dma_start(arin[:], kvpack[:])
    nc.gpsimd.collective_compute(
        "AllReduce", ALU.add, replica_groups=REPLICA_GROUPS,
        ins=[arin.opt()], outs=[arout.opt()])
    # kvred trigger issued NOW so it doesn't queue behind the MLP's
    # weight-stream triggers (sync queue is in-order)
    kvred = cx.act1.tile([128, H * 32], F32, name="kvred", tag="kvred")
    nc.sync.dma_start(kvred[:], arout[:])

    # ---- q (feature-major, fp8 DR over cols [0,768)): only needed after
    # kvred, so emitted after the collective launch -- its matmuls fill
    # the all-reduce window with real work
    q16 = cx.act1.tile([128, KT, TOK], F16, name="q16", tag="q16")
    for fp in range(KT // 2):
        ps = cx.ps()
        for i in range(2):
            ft = 2 * fp + i
            sl = slice(i * TOK, (i + 1) * TOK)
            for kd in range(KT):
                nc.tensor.matmul(ps[:, sl],
                                 w["wqk8"][:, :, kd, ft * 128:(ft + 1) * 128],
                                 zz[:, :, kd, :], start=(kd == 0), stop=False,
                                 perf_mode=mybir.MatmulPerfMode.DoubleRow)
            nc.tensor.matmul(ps[:, sl], w["B3qk"][:, ft * 128:(ft + 1) * 128],
                             R3[:], start=False, stop=True)
        _elu1(nc, cx, ps[:], q16[:, 2 * fp:2 * fp + 2, :], 512)

    # ---- prefetch wout ft-chunks for the attention GEMM
    wos = []
    for ft in range(KT):
        c = cx.stream.tile([128, KT, 128], F16, name="wos", tag="wos")
        nc.sync.dma_start(c[:], seg["wout"].ap()[ft])
        wos.append(c)

    # ---- next frame's x-side: its DVE chain fills the all-reduce window
    xs_next = _x_stage(nc, cx, x_src, nxt) if nxt is not None else None

    # ---- attention block: emitted BEFORE the deferred MLP so it has
    # higher list-scheduler priority and preempts leftover MLP work the
    # moment kvred lands; the MLP (always-ready, lower priority) fills
    # the all-reduce window and h-chain stalls.
    # block-diag kv (f16; KVS already folded pre-AR, wout carries the 256x)
    for hp in range(KT):
        nc.vector.tensor_copy(cx.bd16[0:64, hp, 0:64],
                              kvred[0:64, hp * 64:(hp + 1) * 64])
        nc.vector.tensor_copy(cx.bd16[64:128, hp, 64:128],
                              kvred[64:128, hp * 64:(hp + 1) * 64])
    o16 = cx.act1.tile([128, KT, TOK], F16, name="o16", tag="o16")
    for fp in range(KT // 2):
        ps = cx.ps()
        for i in range(2):
            hp = 2 * fp + i
            nc.tensor.matmul(ps[:, i * TOK:(i + 1) * TOK],
                             cx.bd16[:, hp, :], q16[:, hp, :],
                             start=True, stop=True)
        nc.vector.tensor_copy(o16[:, 2 * fp:2 * fp + 2, :], ps[:])

    # attn (feature-major); (attn+bout) gathered into at32 on DVE,
    # then two wide DVE adds + per-ft pos STT update x2 and heff
    at32 = cx.act1.tile([128, KT, TOK], F32, name="at32", tag="at32")
    for fp in range(KT // 2):
        ps = cx.ps()
        for i in range(2):
            ft = 2 * fp + i
            sl = slice(i * TOK, (i + 1) * TOK)
            for hp in range(KT):
                nc.tensor.matmul(ps[:, sl], wos[ft][:, hp, :],
                                 o16[:, hp, :], start=(hp == 0),
                                 stop=(hp == KT - 1))
        for i in range(2):
            ft = 2 * fp + i
            nc.vector.tensor_scalar_add(at32[:, ft, :],
                                        ps[:, i * TOK:(i + 1) * TOK],
                                        w["bout"][:, ft:ft + 1])
    # x2 = attn + x_eff, in place over xeff (must read at32 before the
    # heff update below overwrites it)
    nc.vector.tensor_add(xeff[:], at32[:], xeff[:])
    if nxt is not None:
        nc.vector.tensor_add(at32[:], at32[:], heff[:])
        for ft in range(KT):
            nc.vector.scalar_tensor_tensor(
                heff[:, ft, :], cx.spat[:, ft, :], cx.tpos[:, ft, tpn:tpn + 1],
                at32[:, ft, :], op0=ALU.mult, op1=ALU.add)

    # ---- output LN -> z2 for the deferred MLP; emitted right after the
    # attention tail so MLP(t) (which fills frame t+1's h-chain stall)
    # becomes ready as early as possible
    # o-chain borrows the h-chain's xq buffer: h-LN(t)'s xq readers are
    # done once zz/z16h are written, well before the attention tail
    ob, rbo, meano, rb32o = _ln_chain(nc, cx, xeff, "o", buf_tag="xb_h")
    R2 = cx.act2.tile([65, TOK], F16, name="R2", tag="R2")
    nc.vector.memset(R2[:], 0.0)     # garbage rows x zero weights else NaN
    nc.vector.memset(R2[0:1, :], 1.0)
    nc.vector.tensor_mul(R2[32:33, :], meano[32:33, :], rb32o[32:33, :])
    z2 = cx.act2.tile([128, KT, TOK], F8, name="z2", tag="z2")
    for kd in range(KT):
        nc.vector.tensor_mul(z2[:, kd, :], ob[:, kd, 0:TOK], rbo[:])

    # ---- deferred MLP of the previous frame (hides the all-reduce)
    if pend is not None:
        _emit_mlp(nc, cx, seg, w, pend)

    pend = dict(t=t, z2=z2, R2=R2, x232=xeff, combine=combine,
                out_dst=out_dst, yf_sc=yf_sc)
    return pend, xs_next


def _emit_mlp(nc, cx, seg, w, pend):
    t, z2, R2, x232 = pend["t"], pend["z2"], pend["R2"], pend["x232"]
    combine, out_dst, yf_sc = pend["combine"], pend["out_dst"], pend["yf_sc"]

    # y2 accumulators pair two ft per PSUM bank (3 banks total)
    yps = [cx.psY.tile([128, 2 * TOK], F32, name="psy", tag="psy")
           for _ in range(KT // 2)]

    def ypsl(ft):
        return yps[ft // 2][:, (ft % 2) * TOK:(ft % 2 + 1) * TOK]

    DR = mybir.MatmulPerfMode.DoubleRow
    for mjp in range(MT // 2):
        # bulk weight streams ride the gpsimd SW-DGE queue so their
        # slot-waits never block the sync queue's latency DMAs
        w2s = cx.stream.tile([128, 2, D], F8, name="w2s", tag="w2s")
        nc.gpsimd.dma_start(w2s[:], seg["w2p"].ap()[mjp])
        y1p = cx.stream.tile([128, 2, TOK], F8, name="y1p", tag="y1p")
        # both mj halves share one psM bank -> a single 512-wide gelu
        # (ACT_TABLE_LOAD is 1283ns; halving gelu count halves the thrash)
        ps = cx.psM.tile([128, 2 * TOK], F32, name="psm", tag="psm")
        for i in range(2):
            mj = 2 * mjp + i
            sl = slice(i * TOK, (i + 1) * TOK)
            for kdp in range(KT // 2):
                nc.tensor.matmul(ps[:, sl],
                                 w["g1"][:, mj, 2 * kdp:2 * kdp + 2, :],
                                 z2[:, 2 * kdp:2 * kdp + 2, :],
                                 start=(kdp == 0), stop=False, perf_mode=DR)
            nc.tensor.matmul(ps[:, sl], w["B2"][:, mj * 128:(mj + 1) * 128],
                             R2[:], start=False, stop=True)
        nc.scalar.activation(y1p[:, :, :], ps[:], AF.Gelu, scale=W8SI)
        for ft in range(KT):
            nc.tensor.matmul(ypsl(ft), w2s[:, :, ft * 128:(ft + 1) * 128],
                             y1p[:], start=(mjp == 0), stop=(mjp == MT // 2 - 1),
                             perf_mode=DR)

    if not combine:
        for ft in range(KT):
            yb = cx.tmp.tile([128, TOK], F32, name="yb", tag="yb")
            nc.vector.tensor_scalar(yb[:], ypsl(ft), W8SI,
                                    w["b2"][:, ft:ft + 1],
                                    op0=ALU.mult, op1=ALU.add)
            nc.vector.tensor_add(x232[:, ft, :], yb[:], x232[:, ft, :])
    else:
        yf = cx.act1.tile([128, KT, TOK], F32, name="yfld", tag="yfld")
        nc.sync.dma_start(yf[:], yf_sc[t])
        for ft in range(KT):
            yb = cx.tmp.tile([128, TOK], F32, name="yb", tag="yb")
            nc.vector.tensor_scalar(yb[:], ypsl(ft), W8SI,
                                    w["b2"][:, ft:ft + 1],
                                    op0=ALU.mult, op1=ALU.add)
            nc.vector.tensor_add(yb[:], yb[:], yf[:, ft, :])
            nc.vector.tensor_add(x232[:, ft, :], yb[:], x232[:, ft, :])
    nc.sync.dma_start(out_dst[t], x232[:])


# ---------------------------------------------------------------- entry point

@functools.cache
def _compiled_nc():
    return build_nc()


def kernel(**inputs):
    inputs = {k: np.asarray(v) for k, v in inputs.items()}
    nc = _compiled_nc()
    in_maps = make_in_maps(inputs)
    res = run_bass_kernel_spmd(nc, in_maps, list(range(NCORES)))
    return unshard_output(res.results)


# revision 48
# speedup vs baseline: 1.1848x; 1.0503x over previous
"""Trainium2 Bass kernel for nn_GPTrack2D (dense transformer with linear
attention and a per-frame recurrence over L).

Sharding: batch (2) -> two groups of 4 cores; tokens (1024 -> 256/core)
within each group. Linear attention's k^T v state is all-reduced per frame
within the group; the all-reduce hides behind the previous frame's MLP and
the next frame's x-side LayerNorm (software-pipelined emission).

v3 over v2: fp8(e4m3) DoubleRow matmuls where numerics allow.
- q,k projections: fp8 DR pairing the x-side and h-side contractions
  (zz tile holds fp8 zx/zh planes; wqk8 holds both weight planes, x64).
- v projection stays f16 (fp8 v decorrelates the recurrence: sim showed
  6.8% rel err with v fp8 vs 0.57% with v f16).
- MLP: both GEMMs fp8 DR (g1 pairs kd, w2 pairs mj); numerically free.
- elu+1 on a x64-scaled psum via  max(x,0) + min(exp(x),1)  keeps the
  3-op structure with the 1/64 descale folded into ScalarE's scale arg.
- B2 bias rows padded 33->65 (rank-33 matmuls measured 237ns vs 134).
- Identity copies moved from ScalarE to DVE (ACT_TABLE_LOAD thrash:
  1283ns per function switch, was ~21/frame).
- wqk8/wv16 double-buffered; next segment prefetched mid-scan on the
  gpsimd queue, so scan boundaries don't stall on weight DMA.
- g1 fully SBUF-resident in fp8 (no streamed half).

Precision: residual stream / carry f32; LN stat inputs bf16 (|h| reaches
~1.3e6). kv state scaled by 1/256 to fit f16; wout pre-scaled by 256.
fp8 weights pre-scaled by 64 (0.02-scale weights underflow e4m3 norms).
"""

import functools

import numpy as np
import ml_dtypes

import concourse.bacc as bacc
import concourse.mybir as mybir
from concourse import tile
from concourse.bass_utils import run_bass_kernel_spmd

F32 = mybir.dt.float32
BF16 = mybir.dt.bfloat16
F16 = mybir.dt.float16
F8 = mybir.dt.float8e4
NP_F8 = ml_dtypes.float8_e4m3
AF = mybir.ActivationFunctionType
ALU = mybir.AluOpType

B, L, N, D, M, H = 2, 12, 1024, 768, 3072, 12
NCORES = 8
GROUP = 4                 # cores per batch group
TOK = N // GROUP          # 256 tokens per core
KT = D // 128             # 6 d-tiles
MT = M // 128             # 24 m-tiles
F3 = 3 * D                # 2304
EPS = 1e-5
KVS = 1.0 / 256.0         # kv-state scale so fp16 holds it
KVSI = 256.0
W8S = 64.0                # fp8 weight pre-scale
W8SI = 1.0 / 64.0

# dev-scale knobs (full problem: L_RUN=12, LAYERS_RUN=2, DIRS_RUN=(0, 1))
L_RUN = L
LAYERS_RUN = 2
DIRS_RUN = (0, 1)

REPLICA_GROUPS = [[0, 1, 2, 3], [4, 5, 6, 7]]


# ---------------------------------------------------------------- host prep

def _pack_weights(inputs):
    segs = []
    for layer in range(LAYERS_RUN):
        for d in DIRS_RUN:
            gi = np.asarray(inputs["lni_g"][d, layer]); bi = np.asarray(inputs["lni_b"][d, layer])
            gh = np.asarray(inputs["lnh_g"][d, layer]); bh = np.asarray(inputs["lnh_b"][d, layer])
            go = np.asarray(inputs["lno_g"][d, layer]); bo = np.asarray(inputs["lno_b"][d, layer])
            Wqkv = np.asarray(inputs["Wqkv"][d, layer]); bqkv = np.asarray(inputs["bqkv"][d, layer])
            Wqkvh = np.asarray(inputs["Wqkvh"][d, layer]); bqkvh = np.asarray(inputs["bqkvh"][d, layer])
            Wout = np.asarray(inputs["Wout"][d, layer]); bout = np.asarray(inputs["bout"][d, layer])
            W1 = np.asarray(inputs["W1"][d, layer]); b1 = np.asarray(inputs["b1"][d, layer])
            W2 = np.asarray(inputs["W2"][d, layer]); b2 = np.asarray(inputs["b2"][d, layer])

            gqkv = gi[:, None] * Wqkv                      # (D, 3D)
            gqkvh = gh[:, None] * Wqkvh
            cqkv = bi @ Wqkv + bqkv + bh @ Wqkvh + bqkvh   # (3D,)
            # [p, side, kd, f]: side 0 = x-weights, 1 = h-weights
            wpair = np.stack([
                gqkv.reshape(KT, 128, F3).transpose(1, 0, 2),
                gqkvh.reshape(KT, 128, F3).transpose(1, 0, 2)], axis=1)
            # q,k columns [0, 2D) in fp8 x64; v columns [2D, 3D) in f16
            wqk8 = np.ascontiguousarray(wpair[:, :, :, :2 * D] * W8S).astype(NP_F8)
            wv16 = np.ascontiguousarray(wpair[:, :, :, 2 * D:]).astype(np.float16)
            # bias rows on partitions 0/32/64 (DVE base-partition rule)
            B3 = np.zeros((65, F3), np.float32)
            B3[0], B3[32], B3[64] = cqkv, -gqkv.sum(0), -gqkvh.sum(0)
            B3qk = (B3[:, :2 * D] * W8S).astype(np.float16)
            B3v = B3[:, 2 * D:].astype(np.float16)

            g1 = go[:, None] * W1                          # (D, M)
            c1 = bo @ W1 + b1                              # (M,)
            B2 = np.zeros((65, M), np.float32)
            B2[0], B2[32] = c1 * W8S, -g1.sum(0) * W8S

            seg = dict(
                wqk8=wqk8, wv16=wv16, B3qk=B3qk, B3v=B3v,
                # wout pre-scaled by KVSI: cancels the f16 kv-state 1/256.
                # ft-chunked for streaming: [ft, p, kd, c] = w[kd*128+p, ft*128+c]
                wout=np.ascontiguousarray(
                    (Wout * KVSI).reshape(KT, 128, KT, 128)
                    .transpose(2, 1, 0, 3)).astype(np.float16),
                bout=np.ascontiguousarray(
                    bout.reshape(KT, 128).T).astype(np.float32),
                # fully resident: [p, mj, kd, c] = g1[kd*128+p, mj*128+c]
                g1=np.ascontiguousarray(
                    (g1 * W8S).reshape(KT, 128, MT, 128)
                    .transpose(1, 2, 0, 3)).astype(NP_F8),
                B2=B2.astype(np.float16),
                # mj-paired for DR: [mjp, p, i, f] = W2[(2mjp+i)*128+p, f]
                w2p=np.ascontiguousarray(
                    (W2 * W8S).reshape(MT // 2, 2, 128, D)
                    .transpose(0, 2, 1, 3)).astype(NP_F8),
                b2=np.ascontiguousarray(
                    b2.reshape(KT, 128).T).astype(np.float32),    # (128, KT)
            )
            segs.append(seg)
    return segs


def _feat_major(a, dtype):
    """(..., tok, D) -> (..., 128, KT, tok) tiled feature-major."""
    t = np.moveaxis(np.asarray(a), -1, -2)                # (..., D, tok)
    shp = t.shape[:-2]
    t = t.reshape(shp + (KT, 128, t.shape[-1]))           # (..., KT, 128, tok)
    t = np.moveaxis(t, -3, -2)                            # (..., 128, KT, tok)
    return np.ascontiguousarray(t).astype(dtype)


def make_in_maps(inputs):
    segs = _pack_weights(inputs)
    in_maps = []
    for core in range(NCORES):
        b = core // GROUP
        s = (core % GROUP) * TOK
        m = {}
        m["x_in"] = _feat_major(
            np.asarray(inputs["x"])[b, :L_RUN, s:s + TOK, :], np.float32)
        m["h0_in"] = _feat_major(
            np.asarray(inputs["hidden"])[b, s:s + TOK, :], np.float32)
        m["spat"] = _feat_major(
            np.asarray(inputs["spatial_pos"])[b, s:s + TOK, :], np.float32)
        tp = np.asarray(inputs["temporal_pos"])[b, :L_RUN, :]   # (L, D)
        tp = tp.T.reshape(KT, 128, L_RUN).transpose(1, 0, 2)
        m["tpos"] = np.ascontiguousarray(tp).astype(np.float32)  # (128, KT, L)
        for si, seg in enumerate(segs):
            for k, v in seg.items():
                m[f"{k}_{si}"] = v
        in_maps.append(m)
    return in_maps


def unshard_output(results):
    out = np.empty((B, L_RUN, N, D), np.float32)
    for core in range(NCORES):
        b = core // GROUP
        s = (core % GROUP) * TOK
        o = np.asarray(results[core]["out_x"])            # (L, 128, KT, TOK)
        o = o.transpose(0, 2, 1, 3).reshape(L_RUN, D, TOK)
        out[b, :, s:s + TOK, :] = np.moveaxis(o, -1, -2)
    return out


# ---------------------------------------------------------------- kernel build

class Ctx:
    """Pools, constants and persistent tiles used during emission."""

    def ps(self):
        return self.psA.tile([128, 2 * TOK], F32, name="ps", tag="ps")


def _ln_chain(nc, cx, src32, tag, veng=None, buf_tag=None):
    """Feature-major LN for an SBUF (128, KT, TOK) f32 tile.

    Emits: bf16 copy xb, squares, packed stats matmuls (s1|s2 in one
    PSUM bank), mean/var smalls, sqrt + fast reciprocal. Returns (xb,
    rbb, mean, rb32). veng places the wide copy/square ops (gpsimd for
    off-critical chains keeps the DVE queue short).
    """
    veng = veng or nc.vector
    # xq packs [bf16 copy | its square] so one 512-wide matmul per kd
    # yields both stat sums; squares off ScalarE keep its LUT unthrashed
    xq = cx.act1.tile([128, KT, 2 * TOK], BF16, name=f"xb_{tag}",
                      tag=buf_tag or f"xb_{tag}")
    s12 = cx.psS.tile([128, 2 * TOK], F32, name="s12", tag="s12")
    for kd in range(KT):
        veng.tensor_copy(xq[:, kd, 0:TOK], src32[:, kd, :])
        veng.tensor_mul(xq[:, kd, TOK:2 * TOK], xq[:, kd, 0:TOK],
                        xq[:, kd, 0:TOK])
        nc.tensor.matmul(s12[:], cx.onesB[:], xq[:, kd, :],
                         start=(kd == 0), stop=(kd == KT - 1))
    mean = cx.sm.tile([128, TOK], F32, name="mean", tag="lnsm")
    nc.vector.tensor_scalar_mul(mean[:], s12[:, 0:TOK], 1.0 / D)
    msq = cx.sm.tile([128, TOK], F32, name="msq", tag="lnsm")
    nc.vector.tensor_mul(msq[:], mean[:], mean[:])
    ve = cx.sm.tile([128, TOK], F32, name="ve", tag="lnsm")
    nc.vector.scalar_tensor_tensor(ve[:], s12[:, TOK:2 * TOK], 1.0 / D, msq[:],
                                   op0=ALU.mult, op1=ALU.subtract)
    rb32 = cx.sm.tile([128, TOK], F32, name="rb32", tag="lnsm")
    # 1/sqrt(|ve|+eps) in one ScalarE op (Rsqrt is banned; this isn't)
    nc.scalar.activation(rb32[:], ve[:], AF.Abs_reciprocal_sqrt,
                         bias=cx.epsc[:])
    rbb = cx.tmp.tile([128, TOK], BF16, name=f"rbb_{tag}", tag=f"rbb_{tag}")
    nc.vector.tensor_copy(rbb[:], rb32[:])
    return xq, rbb, mean, rb32


def _elu1(nc, cx, psum_ap, out_ap, ncols):
    """out = elu(x)+1 where psum holds 64*x.

    max(x,0) + min(exp(x),1): tlin and texp both read the psum directly
    (no serial DVE->ScalarE chain); ScalarE folds the 1/64 descale.
    """
    tlin = cx.act1.tile([128, 512], F32, name="elin", tag="elin")
    texp = cx.act1.tile([128, 512], F32, name="eexp", tag="eexp")
    nc.vector.tensor_scalar(tlin[:, :ncols], psum_ap, W8SI, 0.0,
                            op0=ALU.mult, op1=ALU.max)
    nc.scalar.activation(texp[:, :ncols], psum_ap, AF.Exp, scale=W8SI)
    nc.vector.scalar_tensor_tensor(out_ap, texp[:, :ncols], 1.0,
                                   tlin[:, :ncols], op0=ALU.min, op1=ALU.add)


def build_nc():
    nc = bacc.Bacc("TRN2", target_bir_lowering=False, debug=False,
                   num_devices=NCORES)

    x_in = nc.dram_tensor("x_in", [L_RUN, 128, KT, TOK], F32, kind="ExternalInput")
    h0_in = nc.dram_tensor("h0_in", [128, KT, TOK], F32, kind="ExternalInput")
    spat = nc.dram_tensor("spat", [128, KT, TOK], F32, kind="ExternalInput")
    tpos = nc.dram_tensor("tpos", [128, KT, L_RUN], F32, kind="ExternalInput")
    nseg = LAYERS_RUN * len(DIRS_RUN)
    segs = []
    for si in range(nseg):
        segs.append(dict(
            wqk8=nc.dram_tensor(f"wqk8_{si}", [128, 2, KT, 2 * D], F8, kind="ExternalInput"),
            wv16=nc.dram_tensor(f"wv16_{si}", [128, 2, KT, D], F16, kind="ExternalInput"),
            B3qk=nc.dram_tensor(f"B3qk_{si}", [65, 2 * D], F16, kind="ExternalInput"),
            B3v=nc.dram_tensor(f"B3v_{si}", [65, D], F16, kind="ExternalInput"),
            wout=nc.dram_tensor(f"wout_{si}", [KT, 128, KT, 128], F16, kind="ExternalInput"),
            bout=nc.dram_tensor(f"bout_{si}", [128, KT], F32, kind="ExternalInput"),
            g1=nc.dram_tensor(f"g1_{si}", [128, MT, KT, 128], F8, kind="ExternalInput"),
            B2=nc.dram_tensor(f"B2_{si}", [65, M], F16, kind="ExternalInput"),
            w2p=nc.dram_tensor(f"w2p_{si}", [MT // 2, 128, 2, D], F8, kind="ExternalInput"),
            b2=nc.dram_tensor(f"b2_{si}", [128, KT], F32, kind="ExternalInput"),
        ))
    out_x = nc.dram_tensor("out_x", [L_RUN, 128, KT, TOK], F32, kind="ExternalOutput")

    with tile.TileContext(nc) as tc:
        with (
            tc.tile_pool(name="cst", bufs=1) as cst,
            tc.tile_pool(name="wt", bufs=2) as wt,
            tc.tile_pool(name="wt1", bufs=1) as wt1,
            tc.tile_pool(name="wg", bufs=1) as wg,
            tc.tile_pool(name="stream", bufs=3) as stream,
            tc.tile_pool(name="act1", bufs=1) as act1,
            tc.tile_pool(name="act2", bufs=2) as act2,
            tc.tile_pool(name="state", bufs=1) as state,
            tc.tile_pool(name="tmp", bufs=2) as tmp,
            tc.tile_pool(name="sm", bufs=5) as sm,
            tc.tile_pool(name="psA", bufs=2, space="PSUM") as psA,
            tc.tile_pool(name="psS", bufs=1, space="PSUM") as psS,
            tc.tile_pool(name="psM", bufs=2, space="PSUM") as psM,
            tc.tile_pool(name="psY", bufs=3, space="PSUM") as psY,
            tc.tile_pool(name="dram", bufs=2, space="DRAM") as dram,
        ):
            cx = Ctx()
            cx.wt, cx.wt1, cx.wg = wt, wt1, wg
            cx.stream, cx.act1, cx.act2 = stream, act1, act2
            cx.state, cx.tmp, cx.sm = state, tmp, sm
            cx.psA, cx.psS, cx.psM = psA, psS, psM
            cx.psY, cx.dram = psY, dram

            cx.onesB = cst.tile([128, 128], BF16, name="onesB")
            nc.vector.memset(cx.onesB[:], 1.0)
            cx.epsc = cst.tile([128, 1], F32, name="epsc")
            nc.vector.memset(cx.epsc[:], EPS)
            cx.spat = cst.tile([128, KT, TOK], F32, name="spatc")
            nc.sync.dma_start(cx.spat[:], spat.ap())
            cx.tpos = cst.tile([128, KT, L_RUN], F32, name="tposc")
            nc.sync.dma_start(cx.tpos[:], tpos.ap())
            cx.h0_in = h0_in

            x1_sc = dram.tile([L_RUN, 128, KT, TOK], F32, name="x1_sc", tag="x1_sc")

            # layer 1 runs (bwd, fwd): layer 0's SECOND scan (bwd) finishes
            # frame 11 first, which is exactly the frame layer-1-bwd needs
            # first -> the layer boundary pipelines instead of serializing.
            scan_list = []
            for layer in range(LAYERS_RUN):
                dirs = DIRS_RUN if layer == 0 else tuple(reversed(DIRS_RUN))
                for scan_i, d in enumerate(dirs):
                    scan_list.append((layer, scan_i, d))
            w_cur = None
            for order, (layer, scan_i, d) in enumerate(scan_list):
                si = layer * len(DIRS_RUN) + DIRS_RUN.index(d)
                if order == 0:
                    w_cur = _load_wqk8(nc, cx, segs[si], nc.sync)
                    w_cur.update(_load_seg_rest(nc, cx, segs[si], nc.sync))
                if scan_i == 0:
                    yf_sc = dram.tile([L_RUN, 128, KT, TOK], F32,
                                      name="yf_sc", tag="yf_sc")
                x_src = x_in.ap() if layer == 0 else x1_sc
                last_layer = layer == LAYERS_RUN - 1
                fwd = d == 0
                combine = scan_i == 1
                frames = (list(range(L_RUN)) if fwd
                          else list(range(L_RUN - 1, -1, -1)))
                if not combine:
                    out_dst = yf_sc
                elif last_layer:
                    out_dst = out_x.ap()
                else:
                    out_dst = x1_sc
                nsi = None
                if order + 1 < len(scan_list):
                    nl, _, nd = scan_list[order + 1]
                    nsi = nl * len(DIRS_RUN) + DIRS_RUN.index(nd)
                w_next = _emit_scan(nc, cx, segs[si], w_cur,
                                    segs[nsi] if nsi is not None else None,
                                    x_src, frames,
                                    pos_fixed=(layer if fwd else None),
                                    yf_sc=yf_sc, combine=combine,
                                    out_dst=out_dst)
                if nsi is not None:
                    # single-buffered tiles must load AFTER this scan's
                    # last emitted reader (emission order = data order)
                    w_next.update(_load_seg_rest(nc, cx, segs[nsi],
                                                 nc.gpsimd))
                    w_cur = w_next
    nc.compile()
    return nc


def _load_wqk8(nc, cx, seg, eng):
    """wqk8 is double-buffered so it can prefetch mid-scan (the k MMs are
    the first PE work of a scan)."""
    t = cx.wt.tile([128, 2, KT, 2 * D], F8, name="wqk8", tag="wqk8")
    eng.dma_start(t[:], seg["wqk8"].ap())
    return {"wqk8": t}


def _load_seg_rest(nc, cx, seg, eng):
    """Single-buffered weights: must be emitted after the previous
    segment's last reader (Tile orders by emission), i.e. at scan
    boundaries; all are needed >=7us into the scan so the DMA hides."""
    w = {}
    for nm, shape, dt, pool in (
            ("wv16", [128, 2, KT, D], F16, cx.wt1),
            ("B3qk", [65, 2 * D], F16, cx.wt1),
            ("B3v", [65, D], F16, cx.wt1),
            ("B2", [65, M], F16, cx.wt1),
            ("bout", [128, KT], F32, cx.wt1),
            ("b2", [128, KT], F32, cx.wt1),
            ("g1", [128, MT, KT, 128], F8, cx.wg)):
        w[nm] = pool.tile(shape, dt, name=nm, tag=nm)
        eng.dma_start(w[nm][:], seg[nm].ap())
    return w


def _emit_scan(nc, cx, seg, w, seg_next, x_src, frames, pos_fixed, yf_sc,
               combine, out_dst):
    # heff = h0 + pos[tp0] (f32 carry, maintained by the attention tail);
    # h0 borrows the yfld slot (idle at scan starts)
    h0t = cx.act1.tile([128, KT, TOK], F32, name="yfld", tag="yfld")
    nc.sync.dma_start(h0t[:], cx.h0_in.ap())
    # heff/bd16 rotate 2 buffers so the next scan's head (x-stage, h-LN,
    # k/v, its first all-reduce) overlaps this scan's tail frames
    heff = cx.act2.tile([128, KT, TOK], F32, name="heff", tag="heff")
    cx.bd16 = cx.wt1.tile([128, KT, 128], F16, name="bd16", tag="bd16")
    nc.vector.memset(cx.bd16[:], 0.0)  # off-diag blocks must stay zero
    tp0 = pos_fixed if pos_fixed is not None else frames[0]
    for kd in range(KT):
        nc.vector.scalar_tensor_tensor(
            heff[:, kd, :], cx.spat[:, kd, :], cx.tpos[:, kd, tp0:tp0 + 1],
            h0t[:, kd, :], op0=ALU.mult, op1=ALU.add)

    xs = _x_stage(nc, cx, x_src, frames[0])
    pend = None
    w_next = None
    for i, t in enumerate(frames):
        nxt = frames[i + 1] if i + 1 < len(frames) else None
        pend, xs = _emit_frame(nc, cx, seg, w, t, nxt, x_src, heff, xs,
                               pos_fixed, yf_sc, combine, out_dst, pend)
        if i == 1 and seg_next is not None:
            w_next = _load_wqk8(nc, cx, seg_next, nc.gpsimd)
    _emit_mlp(nc, cx, seg, w, pend)
    return w_next


def _x_stage(nc, cx, x_src, t):
    """x-side work for frame t: load, add pos, LN stats, normalize.

    Returns dict(xeff, zz, z16x, R3); zz plane 0 (x-side fp8) and z16x
    (f16 for the v matmuls) are written here; zz plane 1 / z16h by the
    h-stage. R3 rows [ones; mrb_x; <mrb_h>] (row 64 filled by h-stage).
    """
    # xeff doubles as x2 later (attn tail adds in place); lives until the
    # deferred MLP tail of this frame -> 3 buffers (stream pool)
    # rides the gpsimd bulk queue: the sync queue must stay short so the
    # kvred trigger's collective-wait never blocks other transfers
    xeff = cx.stream.tile([128, KT, TOK], F32, name="xe", tag="xe")
    nc.gpsimd.dma_start(xeff[:], x_src[t])
    for kd in range(KT):
        nc.vector.scalar_tensor_tensor(
            xeff[:, kd, :], cx.spat[:, kd, :], cx.tpos[:, kd, t:t + 1],
            xeff[:, kd, :], op0=ALU.mult, op1=ALU.add)
    xb, rbb, mean, rb32 = _ln_chain(nc, cx, xeff, "x")
    R3 = cx.act2.tile([65, TOK], F16, name="R3", tag="R3")
    nc.vector.memset(R3[:], 0.0)     # garbage rows x zero weights else NaN
    nc.vector.memset(R3[0:1, :], 1.0)
    # stats are partition-replicated; read partition 32 to write row 32
    nc.vector.tensor_mul(R3[32:33, :], mean[32:33, :], rb32[32:33, :])
    zz = cx.act2.tile([128, 2, KT, TOK], F8, name="zz", tag="zz")
    z16x = cx.act2.tile([128, KT, TOK], F16, name="z16x", tag="z16x")
    for kd in range(KT):
        nc.vector.tensor_mul(zz[:, 0, kd, :], xb[:, kd, 0:TOK], rbb[:])
        nc.vector.tensor_mul(z16x[:, kd, :], xb[:, kd, 0:TOK], rbb[:])
    return dict(xeff=xeff, zz=zz, z16x=z16x, R3=R3)


def _emit_frame(nc, cx, seg, w, t, nxt, x_src, heff, xs, pos_fixed, yf_sc,
                combine, out_dst, pend):
    tpn = pos_fixed if pos_fixed is not None else nxt   # next frame's pos idx

    xeff, zz, z16x, R3 = xs["xeff"], xs["zz"], xs["z16x"], xs["R3"]

    # ---- h-side LN (critical chain)
    hb, rbh, meanh, rb32h = _ln_chain(nc, cx, heff, "h")
    nc.vector.tensor_mul(R3[64:65, :], meanh[64:65, :], rb32h[64:65, :])
    z16h = cx.act1.tile([128, KT, TOK], F16, name="z16h", tag="z16h")
    for kd in range(KT):
        nc.vector.tensor_mul(zz[:, 1, kd, :], hb[:, kd, 0:TOK], rbh[:])
        nc.vector.tensor_mul(z16h[:, kd, :], hb[:, kd, 0:TOK], rbh[:])

    # ---- k (fp8 DR, cols [D,2D) of qkv = [768,1536) of wqk8) and
    #      v (f16, wv16) -- token-major: (128, 2, D) each [tok-half, feature]
    # 512-wide chunks get a full psum bank; the two 256-wide tails share
    # one bank so every bank carries ~2x MM work per elu/copy hold (a
    # 2-buffer rotation starves when the reader latency exceeds the MM
    # group time).
    k16 = cx.act1.tile([128, 2, D], F16, name="k16", tag="k16")
    v16 = cx.act1.tile([128, 2, D], F16, name="v16", tag="v16")

    def kmm(ps_ap, tok2, lo, nc_):
        tsl = slice(tok2 * 128, (tok2 + 1) * 128)
        for kd in range(KT):
            nc.tensor.matmul(ps_ap, zz[:, :, kd, tsl],
                             w["wqk8"][:, :, kd, lo:lo + nc_],
                             start=(kd == 0), stop=False,
                             perf_mode=mybir.MatmulPerfMode.DoubleRow)
        nc.tensor.matmul(ps_ap, R3[:, tsl],
                         w["B3qk"][:, lo:lo + nc_], start=False, stop=True)

    def vmm(ps_ap, tok2, lo, nc_):
        tsl = slice(tok2 * 128, (tok2 + 1) * 128)
        for kd in range(KT):
            nc.tensor.matmul(ps_ap, z16x[:, kd, tsl],
                             w["wv16"][:, 0, kd, lo:lo + nc_],
                             start=(kd == 0), stop=False)
        for kd in range(KT):
            nc.tensor.matmul(ps_ap, z16h[:, kd, tsl],
                             w["wv16"][:, 1, kd, lo:lo + nc_],
                             start=False, stop=False)
        nc.tensor.matmul(ps_ap, R3[:, tsl],
                         w["B3v"][:, lo:lo + nc_], start=False, stop=True)

    for tok2 in range(2):
        ps = cx.ps()
        kmm(ps[:, 0:512], tok2, D, 512)
        _elu1(nc, cx, ps[:, 0:512], k16[:, tok2, 0:512], 512)
    ps = cx.ps()
    for tok2 in range(2):
        kmm(ps[:, tok2 * TOK:(tok2 + 1) * TOK], tok2, D + 512, 256)
    _elu1(nc, cx, ps[:], k16[:, :, 512:768], 512)
    for tok2 in range(2):
        ps = cx.ps()
        vmm(ps[:, 0:512], tok2, 0, 512)
        nc.vector.tensor_copy(v16[:, tok2, 0:512], ps[:, 0:512])
    ps = cx.ps()
    for tok2 in range(2):
        vmm(ps[:, tok2 * TOK:(tok2 + 1) * TOK], tok2, 512, 256)
    nc.vector.tensor_copy(v16[:, :, 512:768], ps[:])

    # ---- kv state per head-pair; pack diag blocks into (128, 384) f32;
    # three head-pairs per psum bank, two strided reads per bank
    kvpack = cx.act1.tile([128, H * 32], F32, name="kvpack", tag="kvpack")
    for bk in range(2):
        ps = cx.ps()
        for j in range(3):
            hp = 3 * bk + j
            for tok2 in range(2):
                nc.tensor.matmul(ps[:, j * 128:j * 128 + 128],
                                 k16[:, tok2, hp * 128:(hp + 1) * 128],
                                 v16[:, tok2, hp * 128:(hp + 1) * 128],
                                 start=(tok2 == 0), stop=(tok2 == 1))
        # KVS fold happens here (pre-AR) so the post-AR bd16 is a plain cast
        for j in range(3):
            hp = 3 * bk + j
            nc.vector.tensor_scalar_mul(kvpack[0:64, hp * 64:(hp + 1) * 64],
                                        ps[0:64, j * 128:j * 128 + 64], KVS)
            nc.vector.tensor_scalar_mul(kvpack[64:128, hp * 64:(hp + 1) * 64],
                                        ps[64:128, j * 128 + 64:j * 128 + 128],
                                        KVS)

    # ---- all-reduce kv within the token-shard group.  Sync-queue order
    # per frame is [arin, wos x6, kvred]: wos has no AR dependency so it
    # drains DURING the collective; kvred sits last so its semaphore
    # wait can't head-of-line-block anything.
    arin = cx.dram.tile([128, H * 32], F32, name="arin", tag="arin")
    arout = cx.dram.tile([128, H * 32], F32, name="arout", tag="arout")
    nc.sync.dma_start(arin[:], kvpack[:])
    nc.gpsimd.collective_compute(
        "AllReduce", ALU.add, replica_groups=REPLICA_GROUPS,
        ins=[arin.opt()], outs=[arout.opt()])
    wos = []
    for ft in range(KT):
        c = cx.stream.tile([128, KT, 128], F16, name="wos", tag="wos")
        nc.sync.dma_start(c[:], seg["wout"].ap()[ft])
        wos.append(c)
    kvred = cx.act1.tile([128, H * 32], F32, name="kvred", tag="kvred")
    nc.sync.dma_start(kvred[:], arout[:])

    # ---- q (feature-major, fp8 DR over cols [0,768)): only needed after
    # kvred, so emitted after the collective launch -- its matmuls fill
    # the all-reduce window with real work
    q16 = cx.act1.tile([128, KT, TOK], F16, name="q16", tag="q16")
    for fp in range(KT // 2):
        ps = cx.ps()
        for i in range(2):
            ft = 2 * fp + i
            sl = slice(i * TOK, (i + 1) * TOK)
            for kd in range(KT):
                nc.tensor.matmul(ps[:, sl],
                                 w["wqk8"][:, :, kd, ft * 128:(ft + 1) * 128],
                                 zz[:, :, kd, :], start=(kd == 0), stop=False,
                                 perf_mode=mybir.MatmulPerfMode.DoubleRow)
            nc.tensor.matmul(ps[:, sl], w["B3qk"][:, ft * 128:(ft + 1) * 128],
                             R3[:], start=False, stop=True)
        _elu1(nc, cx, ps[:], q16[:, 2 * fp:2 * fp + 2, :], 512)

    # ---- next frame's x-side: its DVE chain fills the all-reduce window
    xs_next = _x_stage(nc, cx, x_src, nxt) if nxt is not None else None

    # ---- attention block: emitted BEFORE the deferred MLP so it has
    # higher list-scheduler priority and preempts leftover MLP work the
    # moment kvred lands; the MLP (always-ready, lower priority) fills
    # the all-reduce window and h-chain stalls.
    # block-diag kv (f16; KVS already folded pre-AR, wout carries the 256x)
    for hp in range(KT):
        nc.vector.tensor_copy(cx.bd16[0:64, hp, 0:64],
                              kvred[0:64, hp * 64:(hp + 1) * 64])
        nc.vector.tensor_copy(cx.bd16[64:128, hp, 64:128],
                              kvred[64:128, hp * 64:(hp + 1) * 64])
    o16 = cx.act1.tile([128, KT, TOK], F16, name="o16", tag="o16")
    for fp in range(KT // 2):
        ps = cx.ps()
        for i in range(2):
            hp = 2 * fp + i
            nc.tensor.matmul(ps[:, i * TOK:(i + 1) * TOK],
                             cx.bd16[:, hp, :], q16[:, hp, :],
                             start=True, stop=True)
        nc.vector.tensor_copy(o16[:, 2 * fp:2 * fp + 2, :], ps[:])

    # attn (feature-major); (attn+bout) gathered into at32 on DVE,
    # then two wide DVE adds + per-ft pos STT update x2 and heff
    at32 = cx.act1.tile([128, KT, TOK], F32, name="at32", tag="at32")
    for fp in range(KT // 2):
        ps = cx.ps()
        for i in range(2):
            ft = 2 * fp + i
            sl = slice(i * TOK, (i + 1) * TOK)
            for hp in range(KT):
                nc.tensor.matmul(ps[:, sl], wos[ft][:, hp, :],
                                 o16[:, hp, :], start=(hp == 0),
                                 stop=(hp == KT - 1))
        for i in range(2):
            ft = 2 * fp + i
            nc.vector.tensor_scalar_add(at32[:, ft, :],
                                        ps[:, i * TOK:(i + 1) * TOK],
                                        w["bout"][:, ft:ft + 1])
    # x2 = attn + x_eff, in place over xeff (must read at32 before the
    # heff update below overwrites it)
    nc.vector.tensor_add(xeff[:], at32[:], xeff[:])
    if nxt is not None:
        nc.vector.tensor_add(at32[:], at32[:], heff[:])
        for ft in range(KT):
            nc.vector.scalar_tensor_tensor(
                heff[:, ft, :], cx.spat[:, ft, :], cx.tpos[:, ft, tpn:tpn + 1],
                at32[:, ft, :], op0=ALU.mult, op1=ALU.add)

    # ---- output LN -> z2 for the deferred MLP; emitted right after the
    # attention tail so MLP(t) (which fills frame t+1's h-chain stall)
    # becomes ready as early as possible
    # o-chain borrows the h-chain's xq buffer: h-LN(t)'s xq readers are
    # done once zz/z16h are written, well before the attention tail
    ob, rbo, meano, rb32o = _ln_chain(nc, cx, xeff, "o", buf_tag="xb_h")
    R2 = cx.act2.tile([65, TOK], F16, name="R2", tag="R2")
    nc.vector.memset(R2[:], 0.0)     # garbage rows x zero weights else NaN
    nc.vector.memset(R2[0:1, :], 1.0)
    nc.vector.tensor_mul(R2[32:33, :], meano[32:33, :], rb32o[32:33, :])
    z2 = cx.act2.tile([128, KT, TOK], F8, name="z2", tag="z2")
    for kd in range(KT):
        nc.vector.tensor_mul(z2[:, kd, :], ob[:, kd, 0:TOK], rbo[:])

    # ---- deferred MLP of the previous frame (hides the all-reduce)
    if pend is not None:
        _emit_mlp(nc, cx, seg, w, pend)

    pend = dict(t=t, z2=z2, R2=R2, x232=xeff, combine=combine,
                out_dst=out_dst, yf_sc=yf_sc)
    return pend, xs_next


def _emit_mlp(nc, cx, seg, w, pend):
    t, z2, R2, x232 = pend["t"], pend["z2"], pend["R2"], pend["x232"]
    combine, out_dst, yf_sc = pend["combine"], pend["out_dst"], pend["yf_sc"]

    # y2 accumulators pair two ft per PSUM bank (3 banks total)
    yps = [cx.psY.tile([128, 2 * TOK], F32, name="psy", tag="psy")
           for _ in range(KT // 2)]

    def ypsl(ft):
        return yps[ft // 2][:, (ft % 2) * TOK:(ft % 2 + 1) * TOK]

    DR = mybir.MatmulPerfMode.DoubleRow
    for mjp in range(MT // 2):
        # bulk weight streams ride the gpsimd SW-DGE queue so their
        # slot-waits never block the sync queue's latency DMAs
        w2s = cx.stream.tile([128, 2, D], F8, name="w2s", tag="w2s")
        nc.gpsimd.dma_start(w2s[:], seg["w2p"].ap()[mjp])
        y1p = cx.stream.tile([128, 2, TOK], F8, name="y1p", tag="y1p")
        # both mj halves share one psM bank -> a single 512-wide gelu
        # (ACT_TABLE_LOAD is 1283ns; halving gelu count halves the thrash)
        ps = cx.psM.tile([128, 2 * TOK], F32, name="psm", tag="psm")
        for i in range(2):
            mj = 2 * mjp + i
            sl = slice(i * TOK, (i + 1) * TOK)
            for kdp in range(KT // 2):
                nc.tensor.matmul(ps[:, sl],
                                 w["g1"][:, mj, 2 * kdp:2 * kdp + 2, :],
                                 z2[:, 2 * kdp:2 * kdp + 2, :],
                                 start=(kdp == 0), stop=False, perf_mode=DR)
            nc.tensor.matmul(ps[:, sl], w["B2"][:, mj * 128:(mj + 1) * 128],
                             R2[:], start=False, stop=True)
        nc.scalar.activation(y1p[:, :, :], ps[:], AF.Gelu, scale=W8SI)
        for ft in range(KT):
            nc.tensor.matmul(ypsl(ft), w2s[:, :, ft * 128:(ft + 1) * 128],
                             y1p[:], start=(mjp == 0), stop=(mjp == MT // 2 - 1),
                             perf_mode=DR)

    if not combine:
        for ft in range(KT):
            yb = cx.tmp.tile([128, TOK], F32, name="yb", tag="yb")
            nc.vector.tensor_scalar(yb[:], ypsl(ft), W8SI,
                                    w["b2"][:, ft:ft + 1],
                                    op0=ALU.mult, op1=ALU.add)
            nc.vector.tensor_add(x232[:, ft, :], yb[:], x232[:, ft, :])
    else:
        yf = cx.act1.tile([128, KT, TOK], F32, name="yfld", tag="yfld")
        nc.gpsimd.dma_start(yf[:], yf_sc[t])
        for ft in range(KT):
            yb = cx.tmp.tile([128, TOK], F32, name="yb", tag="yb")
            nc.vector.tensor_scalar(yb[:], ypsl(ft), W8SI,
                                    w["b2"][:, ft:ft + 1],
                                    op0=ALU.mult, op1=ALU.add)
            nc.vector.tensor_add(yb[:], yb[:], yf[:, ft, :])
            nc.vector.tensor_add(x232[:, ft, :], yb[:], x232[:, ft, :])
    nc.gpsimd.dma_start(out_dst[t], x232[:])


# ---------------------------------------------------------------- entry point

@functools.cache
def _compiled_nc():
    return build_nc()


def kernel(**inputs):
    inputs = {k: np.asarray(v) for k, v in inputs.items()}
    nc = _compiled_nc()
    in_maps = make_in_maps(inputs)
    res = run_bass_kernel_spmd(nc, in_maps, list(range(NCORES)))
    return unshard_output(res.results)


# revision 49
# speedup vs baseline: 1.2267x; 1.0354x over previous
"""Trainium2 Bass kernel for nn_GPTrack2D (dense transformer with linear
attention and a per-frame recurrence over L).

Sharding: batch (2) -> two groups of 4 cores; tokens (1024 -> 256/core)
within each group. Linear attention's k^T v state is all-reduced per frame
within the group; the all-reduce hides behind the previous frame's MLP and
the next frame's x-side LayerNorm (software-pipelined emission).

v3 over v2: fp8(e4m3) DoubleRow matmuls where numerics allow.
- q,k projections: fp8 DR pairing the x-side and h-side contractions
  (zz tile holds fp8 zx/zh planes; wqk8 holds both weight planes, x64).
- v projection stays f16 (fp8 v decorrelates the recurrence: sim showed
  6.8% rel err with v fp8 vs 0.57% with v f16).
- MLP: both GEMMs fp8 DR (g1 pairs kd, w2 pairs mj); numerically free.
- elu+1 on a x64-scaled psum via  max(x,0) + min(exp(x),1)  keeps the
  3-op structure with the 1/64 descale folded into ScalarE's scale arg.
- B2 bias rows padded 33->65 (rank-33 matmuls measured 237ns vs 134).
- Identity copies moved from ScalarE to DVE (ACT_TABLE_LOAD thrash:
  1283ns per function switch, was ~21/frame).
- wqk8/wv16 double-buffered; next segment prefetched mid-scan on the
  gpsimd queue, so scan boundaries don't stall on weight DMA.
- g1 fully SBUF-resident in fp8 (no streamed half).

Precision: residual stream / carry f32; LN stat inputs bf16 (|h| reaches
~1.3e6). kv state scaled by 1/256 to fit f16; wout pre-scaled by 256.
fp8 weights pre-scaled by 64 (0.02-scale weights underflow e4m3 norms).
"""

import functools

import numpy as np
import ml_dtypes

import concourse.bacc as bacc
import concourse.mybir as mybir
from concourse import tile
from concourse.bass_utils import run_bass_kernel_spmd

F32 = mybir.dt.float32
BF16 = mybir.dt.bfloat16
F16 = mybir.dt.float16
F8 = mybir.dt.float8e4
NP_F8 = ml_dtypes.float8_e4m3
AF = mybir.ActivationFunctionType
ALU = mybir.AluOpType

B, L, N, D, M, H = 2, 12, 1024, 768, 3072, 12
NCORES = 8
GROUP = 4                 # cores per batch group
TOK = N // GROUP          # 256 tokens per core
KT = D // 128             # 6 d-tiles
MT = M // 128             # 24 m-tiles
F3 = 3 * D                # 2304
EPS = 1e-5
KVS = 1.0 / 256.0         # kv-state scale so fp16 holds it
KVSI = 256.0
W8S = 64.0                # fp8 weight pre-scale
W8SI = 1.0 / 64.0

# dev-scale knobs (full problem: L_RUN=12, LAYERS_RUN=2, DIRS_RUN=(0, 1))
L_RUN = L
LAYERS_RUN = 2
DIRS_RUN = (0, 1)

REPLICA_GROUPS = [[0, 1, 2, 3], [4, 5, 6, 7]]


# ---------------------------------------------------------------- host prep

def _pack_weights(inputs):
    segs = []
    for layer in range(LAYERS_RUN):
        for d in DIRS_RUN:
            gi = np.asarray(inputs["lni_g"][d, layer]); bi = np.asarray(inputs["lni_b"][d, layer])
            gh = np.asarray(inputs["lnh_g"][d, layer]); bh = np.asarray(inputs["lnh_b"][d, layer])
            go = np.asarray(inputs["lno_g"][d, layer]); bo = np.asarray(inputs["lno_b"][d, layer])
            Wqkv = np.asarray(inputs["Wqkv"][d, layer]); bqkv = np.asarray(inputs["bqkv"][d, layer])
            Wqkvh = np.asarray(inputs["Wqkvh"][d, layer]); bqkvh = np.asarray(inputs["bqkvh"][d, layer])
            Wout = np.asarray(inputs["Wout"][d, layer]); bout = np.asarray(inputs["bout"][d, layer])
            W1 = np.asarray(inputs["W1"][d, layer]); b1 = np.asarray(inputs["b1"][d, layer])
            W2 = np.asarray(inputs["W2"][d, layer]); b2 = np.asarray(inputs["b2"][d, layer])

            gqkv = gi[:, None] * Wqkv                      # (D, 3D)
            gqkvh = gh[:, None] * Wqkvh
            cqkv = bi @ Wqkv + bqkv + bh @ Wqkvh + bqkvh   # (3D,)
            # [p, side, kd, f]: side 0 = x-weights, 1 = h-weights
            wpair = np.stack([
                gqkv.reshape(KT, 128, F3).transpose(1, 0, 2),
                gqkvh.reshape(KT, 128, F3).transpose(1, 0, 2)], axis=1)
            # q,k columns [0, 2D) in fp8 x64; v columns [2D, 3D) in f16
            wqk8 = np.ascontiguousarray(wpair[:, :, :, :2 * D] * W8S).astype(NP_F8)
            wv16 = np.ascontiguousarray(wpair[:, :, :, 2 * D:]).astype(np.float16)
            # bias rows on partitions 0/32/64 (DVE base-partition rule)
            B3 = np.zeros((65, F3), np.float32)
            B3[0], B3[32], B3[64] = cqkv, -gqkv.sum(0), -gqkvh.sum(0)
            B3qk = (B3[:, :2 * D] * W8S).astype(np.float16)
            B3v = B3[:, 2 * D:].astype(np.float16)

            g1 = go[:, None] * W1                          # (D, M)
            c1 = bo @ W1 + b1                              # (M,)
            B2 = np.zeros((65, M), np.float32)
            B2[0], B2[32] = c1 * W8S, -g1.sum(0) * W8S

            seg = dict(
                wqk8=wqk8, wv16=wv16, B3qk=B3qk, B3v=B3v,
                # wout pre-scaled by KVSI: cancels the f16 kv-state 1/256.
                # ft-chunked for streaming: [ft, p, kd, c] = w[kd*128+p, ft*128+c]
                wout=np.ascontiguousarray(
                    (Wout * KVSI).reshape(KT, 128, KT, 128)
                    .transpose(2, 1, 0, 3)).astype(np.float16),
                bout=np.ascontiguousarray(
                    bout.reshape(KT, 128).T).astype(np.float32),
                # fully resident: [p, mj, kd, c] = g1[kd*128+p, mj*128+c]
                g1=np.ascontiguousarray(
                    (g1 * W8S).reshape(KT, 128, MT, 128)
                    .transpose(1, 2, 0, 3)).astype(NP_F8),
                B2=B2.astype(np.float16),
                # mj-paired for DR: [mjp, p, i, f] = W2[(2mjp+i)*128+p, f]
                w2p=np.ascontiguousarray(
                    (W2 * W8S).reshape(MT // 2, 2, 128, D)
                    .transpose(0, 2, 1, 3)).astype(NP_F8),
                b2=np.ascontiguousarray(
                    b2.reshape(KT, 128).T).astype(np.float32),    # (128, KT)
            )
            segs.append(seg)
    return segs


def _feat_major(a, dtype):
    """(..., tok, D) -> (..., 128, KT, tok) tiled feature-major."""
    t = np.moveaxis(np.asarray(a), -1, -2)                # (..., D, tok)
    shp = t.shape[:-2]
    t = t.reshape(shp + (KT, 128, t.shape[-1]))           # (..., KT, 128, tok)
    t = np.moveaxis(t, -3, -2)                            # (..., 128, KT, tok)
    return np.ascontiguousarray(t).astype(dtype)


def make_in_maps(inputs):
    segs = _pack_weights(inputs)
    in_maps = []
    for core in range(NCORES):
        b = core // GROUP
        s = (core % GROUP) * TOK
        m = {}
        m["x_in"] = _feat_major(
            np.asarray(inputs["x"])[b, :L_RUN, s:s + TOK, :], np.float32)
        m["h0_in"] = _feat_major(
            np.asarray(inputs["hidden"])[b, s:s + TOK, :], np.float32)
        m["spat"] = _feat_major(
            np.asarray(inputs["spatial_pos"])[b, s:s + TOK, :], np.float32)
        tp = np.asarray(inputs["temporal_pos"])[b, :L_RUN, :]   # (L, D)
        tp = tp.T.reshape(KT, 128, L_RUN).transpose(1, 0, 2)
        m["tpos"] = np.ascontiguousarray(tp).astype(np.float32)  # (128, KT, L)
        for si, seg in enumerate(segs):
            for k, v in seg.items():
                m[f"{k}_{si}"] = v
        in_maps.append(m)
    return in_maps


def unshard_output(results):
    out = np.empty((B, L_RUN, N, D), np.float32)
    for core in range(NCORES):
        b = core // GROUP
        s = (core % GROUP) * TOK
        o = np.asarray(results[core]["out_x"])            # (L, 128, KT, TOK)
        o = o.transpose(0, 2, 1, 3).reshape(L_RUN, D, TOK)
        out[b, :, s:s + TOK, :] = np.moveaxis(o, -1, -2)
    return out


# ---------------------------------------------------------------- kernel build

class Ctx:
    """Pools, constants and persistent tiles used during emission."""

    def ps(self):
        return self.psA.tile([128, 2 * TOK], F32, name="ps", tag="ps")


def _ln_chain(nc, cx, src32, tag, veng=None, buf_tag=None):
    """Feature-major LN for an SBUF (128, KT, TOK) f32 tile.

    Emits: bf16 copy xb, squares, packed stats matmuls (s1|s2 in one
    PSUM bank), mean/var smalls, sqrt + fast reciprocal. Returns (xb,
    rbb, mean, rb32). veng places the wide copy/square ops (gpsimd for
    off-critical chains keeps the DVE queue short).
    """
    veng = veng or nc.vector
    # xq packs [bf16 copy | its square] so one 512-wide matmul per kd
    # yields both stat sums; squares off ScalarE keep its LUT unthrashed
    xq = cx.act1.tile([128, KT, 2 * TOK], BF16, name=f"xb_{tag}",
                      tag=buf_tag or f"xb_{tag}")
    s12 = cx.psS.tile([128, 2 * TOK], F32, name="s12", tag="s12")
    for kd in range(KT):
        veng.tensor_copy(xq[:, kd, 0:TOK], src32[:, kd, :])
        veng.tensor_mul(xq[:, kd, TOK:2 * TOK], xq[:, kd, 0:TOK],
                        xq[:, kd, 0:TOK])
        nc.tensor.matmul(s12[:], cx.onesB[:], xq[:, kd, :],
                         start=(kd == 0), stop=(kd == KT - 1))
    mean = cx.sm.tile([128, TOK], F32, name="mean", tag="lnsm")
    nc.vector.tensor_scalar_mul(mean[:], s12[:, 0:TOK], 1.0 / D)
    msq = cx.sm.tile([128, TOK], F32, name="msq", tag="lnsm")
    nc.vector.tensor_mul(msq[:], mean[:], mean[:])
    ve = cx.sm.tile([128, TOK], F32, name="ve", tag="lnsm")
    nc.vector.scalar_tensor_tensor(ve[:], s12[:, TOK:2 * TOK], 1.0 / D, msq[:],
                                   op0=ALU.mult, op1=ALU.subtract)
    rb32 = cx.sm.tile([128, TOK], F32, name="rb32", tag="lnsm")
    # 1/sqrt(|ve|+eps) in one ScalarE op (Rsqrt is banned; this isn't)
    nc.scalar.activation(rb32[:], ve[:], AF.Abs_reciprocal_sqrt,
                         bias=cx.epsc[:])
    rbb = cx.tmp.tile([128, TOK], BF16, name=f"rbb_{tag}", tag=f"rbb_{tag}")
    nc.vector.tensor_copy(rbb[:], rb32[:])
    return xq, rbb, mean, rb32


def _elu1(nc, cx, psum_ap, out_ap, ncols):
    """out = elu(x)+1 where psum holds 64*x.

    max(x,0) + min(exp(x),1): tlin and texp both read the psum directly
    (no serial DVE->ScalarE chain); ScalarE folds the 1/64 descale.
    """
    tlin = cx.act1.tile([128, 512], F32, name="elin", tag="elin")
    texp = cx.act1.tile([128, 512], F32, name="eexp", tag="eexp")
    nc.vector.tensor_scalar(tlin[:, :ncols], psum_ap, W8SI, 0.0,
                            op0=ALU.mult, op1=ALU.max)
    nc.scalar.activation(texp[:, :ncols], psum_ap, AF.Exp, scale=W8SI)
    nc.vector.scalar_tensor_tensor(out_ap, texp[:, :ncols], 1.0,
                                   tlin[:, :ncols], op0=ALU.min, op1=ALU.add)


def build_nc():
    nc = bacc.Bacc("TRN2", target_bir_lowering=False, debug=False,
                   num_devices=NCORES)

    x_in = nc.dram_tensor("x_in", [L_RUN, 128, KT, TOK], F32, kind="ExternalInput")
    h0_in = nc.dram_tensor("h0_in", [128, KT, TOK], F32, kind="ExternalInput")
    spat = nc.dram_tensor("spat", [128, KT, TOK], F32, kind="ExternalInput")
    tpos = nc.dram_tensor("tpos", [128, KT, L_RUN], F32, kind="ExternalInput")
    nseg = LAYERS_RUN * len(DIRS_RUN)
    segs = []
    for si in range(nseg):
        segs.append(dict(
            wqk8=nc.dram_tensor(f"wqk8_{si}", [128, 2, KT, 2 * D], F8, kind="ExternalInput"),
            wv16=nc.dram_tensor(f"wv16_{si}", [128, 2, KT, D], F16, kind="ExternalInput"),
            B3qk=nc.dram_tensor(f"B3qk_{si}", [65, 2 * D], F16, kind="ExternalInput"),
            B3v=nc.dram_tensor(f"B3v_{si}", [65, D], F16, kind="ExternalInput"),
            wout=nc.dram_tensor(f"wout_{si}", [KT, 128, KT, 128], F16, kind="ExternalInput"),
            bout=nc.dram_tensor(f"bout_{si}", [128, KT], F32, kind="ExternalInput"),
            g1=nc.dram_tensor(f"g1_{si}", [128, MT, KT, 128], F8, kind="ExternalInput"),
            B2=nc.dram_tensor(f"B2_{si}", [65, M], F16, kind="ExternalInput"),
            w2p=nc.dram_tensor(f"w2p_{si}", [MT // 2, 128, 2, D], F8, kind="ExternalInput"),
            b2=nc.dram_tensor(f"b2_{si}", [128, KT], F32, kind="ExternalInput"),
        ))
    out_x = nc.dram_tensor("out_x", [L_RUN, 128, KT, TOK], F32, kind="ExternalOutput")

    with tile.TileContext(nc) as tc:
        with (
            tc.tile_pool(name="cst", bufs=1) as cst,
            tc.tile_pool(name="wt", bufs=2) as wt,
            tc.tile_pool(name="wt1", bufs=1) as wt1,
            tc.tile_pool(name="wg", bufs=1) as wg,
            tc.tile_pool(name="stream", bufs=3) as stream,
            tc.tile_pool(name="act1", bufs=1) as act1,
            tc.tile_pool(name="act2", bufs=2) as act2,
            tc.tile_pool(name="state", bufs=1) as state,
            tc.tile_pool(name="tmp", bufs=2) as tmp,
            tc.tile_pool(name="sm", bufs=5) as sm,
            tc.tile_pool(name="psA", bufs=2, space="PSUM") as psA,
            tc.tile_pool(name="psS", bufs=1, space="PSUM") as psS,
            tc.tile_pool(name="psM", bufs=2, space="PSUM") as psM,
            tc.tile_pool(name="psY", bufs=3, space="PSUM") as psY,
            tc.tile_pool(name="dram", bufs=2, space="DRAM") as dram,
        ):
            cx = Ctx()
            cx.wt, cx.wt1, cx.wg = wt, wt1, wg
            cx.stream, cx.act1, cx.act2 = stream, act1, act2
            cx.state, cx.tmp, cx.sm = state, tmp, sm
            cx.psA, cx.psS, cx.psM = psA, psS, psM
            cx.psY, cx.dram = psY, dram

            cx.onesB = cst.tile([128, 128], BF16, name="onesB")
            nc.vector.memset(cx.onesB[:], 1.0)
            cx.epsc = cst.tile([128, 1], F32, name="epsc")
            nc.vector.memset(cx.epsc[:], EPS)
            cx.spat = cst.tile([128, KT, TOK], F32, name="spatc")
            nc.sync.dma_start(cx.spat[:], spat.ap())
            cx.tpos = cst.tile([128, KT, L_RUN], F32, name="tposc")
            nc.sync.dma_start(cx.tpos[:], tpos.ap())
            cx.h0_in = h0_in

            x1_sc = dram.tile([L_RUN, 128, KT, TOK], F32, name="x1_sc", tag="x1_sc")

            # layer 1 runs (bwd, fwd): layer 0's SECOND scan (bwd) finishes
            # frame 11 first, which is exactly the frame layer-1-bwd needs
            # first -> the layer boundary pipelines instead of serializing.
            scan_list = []
            for layer in range(LAYERS_RUN):
                dirs = DIRS_RUN if layer == 0 else tuple(reversed(DIRS_RUN))
                for scan_i, d in enumerate(dirs):
                    scan_list.append((layer, scan_i, d))
            w_cur = None
            for order, (layer, scan_i, d) in enumerate(scan_list):
                si = layer * len(DIRS_RUN) + DIRS_RUN.index(d)
                if order == 0:
                    w_cur = _load_wqk8(nc, cx, segs[si], nc.sync)
                    w_cur.update(_load_seg_rest(nc, cx, segs[si], nc.sync))
                if scan_i == 0:
                    yf_sc = dram.tile([L_RUN, 128, KT, TOK], F32,
                                      name="yf_sc", tag="yf_sc")
                x_src = x_in.ap() if layer == 0 else x1_sc
                last_layer = layer == LAYERS_RUN - 1
                fwd = d == 0
                combine = scan_i == 1
                frames = (list(range(L_RUN)) if fwd
                          else list(range(L_RUN - 1, -1, -1)))
                if not combine:
                    out_dst = yf_sc
                elif last_layer:
                    out_dst = out_x.ap()
                else:
                    out_dst = x1_sc
                nsi = None
                if order + 1 < len(scan_list):
                    nl, _, nd = scan_list[order + 1]
                    nsi = nl * len(DIRS_RUN) + DIRS_RUN.index(nd)
                w_next = _emit_scan(nc, cx, segs[si], w_cur,
                                    segs[nsi] if nsi is not None else None,
                                    x_src, frames,
                                    pos_fixed=(layer if fwd else None),
                                    yf_sc=yf_sc, combine=combine,
                                    out_dst=out_dst)
                if nsi is not None:
                    # single-buffered tiles must load AFTER this scan's
                    # last emitted reader (emission order = data order)
                    w_next.update(_load_seg_rest(nc, cx, segs[nsi],
                                                 nc.gpsimd))
                    w_cur = w_next
    nc.compile()
    return nc


def _load_wqk8(nc, cx, seg, eng):
    """wqk8 is double-buffered so it can prefetch mid-scan (the k MMs are
    the first PE work of a scan)."""
    t = cx.wt.tile([128, 2, KT, 2 * D], F8, name="wqk8", tag="wqk8")
    eng.dma_start(t[:], seg["wqk8"].ap())
    return {"wqk8": t}


def _load_seg_rest(nc, cx, seg, eng):
    """Single-buffered weights: must be emitted after the previous
    segment's last reader (Tile orders by emission), i.e. at scan
    boundaries; all are needed >=7us into the scan so the DMA hides."""
    w = {}
    for nm, shape, dt, pool in (
            ("wv16", [128, 2, KT, D], F16, cx.wt1),
            ("B3qk", [65, 2 * D], F16, cx.wt1),
            ("B3v", [65, D], F16, cx.wt1),
            ("B2", [65, M], F16, cx.wt1),
            ("bout", [128, KT], F32, cx.wt1),
            ("b2", [128, KT], F32, cx.wt1),
            ("g1", [128, MT, KT, 128], F8, cx.wg)):
        w[nm] = pool.tile(shape, dt, name=nm, tag=nm)
        eng.dma_start(w[nm][:], seg[nm].ap())
    return w


def _emit_scan(nc, cx, seg, w, seg_next, x_src, frames, pos_fixed, yf_sc,
               combine, out_dst):
    # heff = h0 + pos[tp0] (f32 carry, maintained by the attention tail);
    # h0 borrows the yfld slot (idle at scan starts)
    h0t = cx.act1.tile([128, KT, TOK], F32, name="yfld", tag="yfld")
    nc.sync.dma_start(h0t[:], cx.h0_in.ap())
    # heff/bd16 rotate 2 buffers so the next scan's head (x-stage, h-LN,
    # k/v, its first all-reduce) overlaps this scan's tail frames
    heff = cx.act2.tile([128, KT, TOK], F32, name="heff", tag="heff")
    cx.bd16 = cx.wt1.tile([128, KT, 128], F16, name="bd16", tag="bd16")
    nc.vector.memset(cx.bd16[:], 0.0)  # off-diag blocks must stay zero
    tp0 = pos_fixed if pos_fixed is not None else frames[0]
    for kd in range(KT):
        nc.vector.scalar_tensor_tensor(
            heff[:, kd, :], cx.spat[:, kd, :], cx.tpos[:, kd, tp0:tp0 + 1],
            h0t[:, kd, :], op0=ALU.mult, op1=ALU.add)

    xs = _x_stage(nc, cx, x_src, frames[0])
    pend = None
    w_next = None
    for i, t in enumerate(frames):
        nxt = frames[i + 1] if i + 1 < len(frames) else None
        pend, xs = _emit_frame(nc, cx, seg, w, t, nxt, x_src, heff, xs,
                               pos_fixed, yf_sc, combine, out_dst, pend)
        if i == 1 and seg_next is not None:
            w_next = _load_wqk8(nc, cx, seg_next, nc.gpsimd)
    _emit_mlp(nc, cx, seg, w, pend)
    return w_next


def _x_stage(nc, cx, x_src, t):
    """x-side work for frame t: load, add pos, LN stats, normalize.

    Returns dict(xeff, zz, z16x, R3); zz plane 0 (x-side fp8) and z16x
    (f16 for the v matmuls) are written here; zz plane 1 / z16h by the
    h-stage. R3 rows [ones; mrb_x; <mrb_h>] (row 64 filled by h-stage).
    """
    # xeff doubles as x2 later (attn tail adds in place); lives until the
    # deferred MLP tail of this frame -> 3 buffers (stream pool)
    # rides the gpsimd bulk queue: the sync queue must stay short so the
    # kvred trigger's collective-wait never blocks other transfers
    xeff = cx.stream.tile([128, KT, TOK], F32, name="xe", tag="xe")
    nc.gpsimd.dma_start(xeff[:], x_src[t])
    for kd in range(KT):
        nc.vector.scalar_tensor_tensor(
            xeff[:, kd, :], cx.spat[:, kd, :], cx.tpos[:, kd, t:t + 1],
            xeff[:, kd, :], op0=ALU.mult, op1=ALU.add)
    xb, rbb, mean, rb32 = _ln_chain(nc, cx, xeff, "x")
    R3 = cx.act2.tile([65, TOK], F16, name="R3", tag="R3")
    nc.vector.memset(R3[:], 0.0)     # garbage rows x zero weights else NaN
    nc.vector.memset(R3[0:1, :], 1.0)
    # stats are partition-replicated; read partition 32 to write row 32
    nc.vector.tensor_mul(R3[32:33, :], mean[32:33, :], rb32[32:33, :])
    zz = cx.act2.tile([128, 2, KT, TOK], F8, name="zz", tag="zz")
    z16x = cx.act2.tile([128, KT, TOK], F16, name="z16x", tag="z16x")
    for kd in range(KT):
        nc.vector.tensor_mul(zz[:, 0, kd, :], xb[:, kd, 0:TOK], rbb[:])
        nc.vector.tensor_mul(z16x[:, kd, :], xb[:, kd, 0:TOK], rbb[:])
    return dict(xeff=xeff, zz=zz, z16x=z16x, R3=R3)


def _emit_frame(nc, cx, seg, w, t, nxt, x_src, heff, xs, pos_fixed, yf_sc,
                combine, out_dst, pend):
    tpn = pos_fixed if pos_fixed is not None else nxt   # next frame's pos idx

    xeff, zz, z16x, R3 = xs["xeff"], xs["zz"], xs["z16x"], xs["R3"]

    # ---- h-side LN (critical chain)
    hb, rbh, meanh, rb32h = _ln_chain(nc, cx, heff, "h")
    nc.vector.tensor_mul(R3[64:65, :], meanh[64:65, :], rb32h[64:65, :])
    z16h = cx.act1.tile([128, KT, TOK], F16, name="z16h", tag="z16h")
    for kd in range(KT):
        nc.vector.tensor_mul(zz[:, 1, kd, :], hb[:, kd, 0:TOK], rbh[:])
        nc.vector.tensor_mul(z16h[:, kd, :], hb[:, kd, 0:TOK], rbh[:])

    # ---- k (fp8 DR, cols [D,2D) of qkv = [768,1536) of wqk8) and
    #      v (f16, wv16) -- token-major: (128, 2, D) each [tok-half, feature]
    # 512-wide chunks get a full psum bank; the two 256-wide tails share
    # one bank so every bank carries ~2x MM work per elu/copy hold (a
    # 2-buffer rotation starves when the reader latency exceeds the MM
    # group time).
    k16 = cx.act1.tile([128, 2, D], F16, name="k16", tag="k16")
    v16 = cx.act1.tile([128, 2, D], F16, name="v16", tag="v16")

    def kmm(ps_ap, tok2, lo, nc_):
        tsl = slice(tok2 * 128, (tok2 + 1) * 128)
        for kd in range(KT):
            nc.tensor.matmul(ps_ap, zz[:, :, kd, tsl],
                             w["wqk8"][:, :, kd, lo:lo + nc_],
                             start=(kd == 0), stop=False,
                             perf_mode=mybir.MatmulPerfMode.DoubleRow)
        nc.tensor.matmul(ps_ap, R3[:, tsl],
                         w["B3qk"][:, lo:lo + nc_], start=False, stop=True)

    def vmm(ps_ap, tok2, lo, nc_):
        tsl = slice(tok2 * 128, (tok2 + 1) * 128)
        for kd in range(KT):
            nc.tensor.matmul(ps_ap, z16x[:, kd, tsl],
                             w["wv16"][:, 0, kd, lo:lo + nc_],
                             start=(kd == 0), stop=False)
        for kd in range(KT):
            nc.tensor.matmul(ps_ap, z16h[:, kd, tsl],
                             w["wv16"][:, 1, kd, lo:lo + nc_],
                             start=False, stop=False)
        nc.tensor.matmul(ps_ap, R3[:, tsl],
                         w["B3v"][:, lo:lo + nc_], start=False, stop=True)

    for tok2 in range(2):
        ps = cx.ps()
        kmm(ps[:, 0:512], tok2, D, 512)
        _elu1(nc, cx, ps[:, 0:512], k16[:, tok2, 0:512], 512)
    ps = cx.ps()
    for tok2 in range(2):
        kmm(ps[:, tok2 * TOK:(tok2 + 1) * TOK], tok2, D + 512, 256)
    _elu1(nc, cx, ps[:], k16[:, :, 512:768], 512)
    for tok2 in range(2):
        ps = cx.ps()
        vmm(ps[:, 0:512], tok2, 0, 512)
        nc.vector.tensor_copy(v16[:, tok2, 0:512], ps[:, 0:512])
    ps = cx.ps()
    for tok2 in range(2):
        vmm(ps[:, tok2 * TOK:(tok2 + 1) * TOK], tok2, 512, 256)
    nc.vector.tensor_copy(v16[:, :, 512:768], ps[:])

    # ---- kv state per head-pair; pack diag blocks into (128, 384) f32;
    # three head-pairs per psum bank, two strided reads per bank
    kvpack = cx.act1.tile([128, H * 32], F16, name="kvpack", tag="kvpack")
    for bk in range(2):
        ps = cx.ps()
        for j in range(3):
            hp = 3 * bk + j
            for tok2 in range(2):
                nc.tensor.matmul(ps[:, j * 128:j * 128 + 128],
                                 k16[:, tok2, hp * 128:(hp + 1) * 128],
                                 v16[:, tok2, hp * 128:(hp + 1) * 128],
                                 start=(tok2 == 0), stop=(tok2 == 1))
        # KVS fold happens here (pre-AR) so the post-AR bd16 is a plain cast
        for j in range(3):
            hp = 3 * bk + j
            nc.vector.tensor_scalar_mul(kvpack[0:64, hp * 64:(hp + 1) * 64],
                                        ps[0:64, j * 128:j * 128 + 64], KVS)
            nc.vector.tensor_scalar_mul(kvpack[64:128, hp * 64:(hp + 1) * 64],
                                        ps[64:128, j * 128 + 64:j * 128 + 128],
                                        KVS)

    # ---- all-reduce kv within the token-shard group.  Sync-queue order
    # per frame is [arin, wos x6, kvred]: wos has no AR dependency so it
    # drains DURING the collective; kvred sits last so its semaphore
    # wait can't head-of-line-block anything.
    arin = cx.dram.tile([128, H * 32], F16, name="arin", tag="arin")
    arout = cx.dram.tile([128, H * 32], F16, name="arout", tag="arout")
    nc.sync.dma_start(arin[:], kvpack[:])
    nc.gpsimd.collective_compute(
        "AllReduce", ALU.add, replica_groups=REPLICA_GROUPS,
        ins=[arin.opt()], outs=[arout.opt()])
    wos = []
    for ft in range(KT):
        c = cx.stream.tile([128, KT, 128], F16, name="wos", tag="wos")
        nc.sync.dma_start(c[:], seg["wout"].ap()[ft])
        wos.append(c)
    kvred = cx.act1.tile([128, H * 32], F16, name="kvred", tag="kvred")
    nc.sync.dma_start(kvred[:], arout[:])

    # ---- q (feature-major, fp8 DR over cols [0,768)): only needed after
    # kvred, so emitted after the collective launch -- its matmuls fill
    # the all-reduce window with real work
    q16 = cx.act1.tile([128, KT, TOK], F16, name="q16", tag="q16")
    for fp in range(KT // 2):
        ps = cx.ps()
        for i in range(2):
            ft = 2 * fp + i
            sl = slice(i * TOK, (i + 1) * TOK)
            for kd in range(KT):
                nc.tensor.matmul(ps[:, sl],
                                 w["wqk8"][:, :, kd, ft * 128:(ft + 1) * 128],
                                 zz[:, :, kd, :], start=(kd == 0), stop=False,
                                 perf_mode=mybir.MatmulPerfMode.DoubleRow)
            nc.tensor.matmul(ps[:, sl], w["B3qk"][:, ft * 128:(ft + 1) * 128],
                             R3[:], start=False, stop=True)
        _elu1(nc, cx, ps[:], q16[:, 2 * fp:2 * fp + 2, :], 512)

    # ---- next frame's x-side: its DVE chain fills the all-reduce window
    xs_next = _x_stage(nc, cx, x_src, nxt) if nxt is not None else None

    # ---- attention block: emitted BEFORE the deferred MLP so it has
    # higher list-scheduler priority and preempts leftover MLP work the
    # moment kvred lands; the MLP (always-ready, lower priority) fills
    # the all-reduce window and h-chain stalls.
    # block-diag kv (f16; KVS already folded pre-AR, wout carries the 256x)
    for hp in range(KT):
        nc.vector.tensor_copy(cx.bd16[0:64, hp, 0:64],
                              kvred[0:64, hp * 64:(hp + 1) * 64])
        nc.vector.tensor_copy(cx.bd16[64:128, hp, 64:128],
                              kvred[64:128, hp * 64:(hp + 1) * 64])
    o16 = cx.act1.tile([128, KT, TOK], F16, name="o16", tag="o16")
    for fp in range(KT // 2):
        ps = cx.ps()
        for i in range(2):
            hp = 2 * fp + i
            nc.tensor.matmul(ps[:, i * TOK:(i + 1) * TOK],
                             cx.bd16[:, hp, :], q16[:, hp, :],
                             start=True, stop=True)
        nc.vector.tensor_copy(o16[:, 2 * fp:2 * fp + 2, :], ps[:])

    # attn (feature-major); (attn+bout) gathered into at32 on DVE,
    # then two wide DVE adds + per-ft pos STT update x2 and heff
    at32 = cx.act1.tile([128, KT, TOK], F32, name="at32", tag="at32")
    for fp in range(KT // 2):
        ps = cx.ps()
        for i in range(2):
            ft = 2 * fp + i
            sl = slice(i * TOK, (i + 1) * TOK)
            for hp in range(KT):
                nc.tensor.matmul(ps[:, sl], wos[ft][:, hp, :],
                                 o16[:, hp, :], start=(hp == 0),
                                 stop=(hp == KT - 1))
        for i in range(2):
            ft = 2 * fp + i
            nc.vector.tensor_scalar_add(at32[:, ft, :],
                                        ps[:, i * TOK:(i + 1) * TOK],
                                        w["bout"][:, ft:ft + 1])
    # x2 = attn + x_eff, in place over xeff (must read at32 before the
    # heff update below overwrites it)
    nc.vector.tensor_add(xeff[:], at32[:], xeff[:])
    if nxt is not None:
        nc.vector.tensor_add(at32[:], at32[:], heff[:])
        for ft in range(KT):
            nc.vector.scalar_tensor_tensor(
                heff[:, ft, :], cx.spat[:, ft, :], cx.tpos[:, ft, tpn:tpn + 1],
                at32[:, ft, :], op0=ALU.mult, op1=ALU.add)

    # ---- output LN -> z2 for the deferred MLP; emitted right after the
    # attention tail so MLP(t) (which fills frame t+1's h-chain stall)
    # becomes ready as early as possible
    # o-chain borrows the h-chain's xq buffer: h-LN(t)'s xq readers are
    # done once zz/z16h are written, well before the attention tail
    ob, rbo, meano, rb32o = _ln_chain(nc, cx, xeff, "o", buf_tag="xb_h")
    R2 = cx.act2.tile([65, TOK], F16, name="R2", tag="R2")
    nc.vector.memset(R2[:], 0.0)     # garbage rows x zero weights else NaN
    nc.vector.memset(R2[0:1, :], 1.0)
    nc.vector.tensor_mul(R2[32:33, :], meano[32:33, :], rb32o[32:33, :])
    z2 = cx.act2.tile([128, KT, TOK], F8, name="z2", tag="z2")
    for kd in range(KT):
        nc.vector.tensor_mul(z2[:, kd, :], ob[:, kd, 0:TOK], rbo[:])

    # ---- deferred MLP of the previous frame (hides the all-reduce)
    if pend is not None:
        _emit_mlp(nc, cx, seg, w, pend)

    pend = dict(t=t, z2=z2, R2=R2, x232=xeff, combine=combine,
                out_dst=out_dst, yf_sc=yf_sc)
    return pend, xs_next


def _emit_mlp(nc, cx, seg, w, pend):
    t, z2, R2, x232 = pend["t"], pend["z2"], pend["R2"], pend["x232"]
    combine, out_dst, yf_sc = pend["combine"], pend["out_dst"], pend["yf_sc"]

    # y2 accumulators pair two ft per PSUM bank (3 banks total)
    yps = [cx.psY.tile([128, 2 * TOK], F32, name="psy", tag="psy")
           for _ in range(KT // 2)]

    def ypsl(ft):
        return yps[ft // 2][:, (ft % 2) * TOK:(ft % 2 + 1) * TOK]

    DR = mybir.MatmulPerfMode.DoubleRow
    for mjp in range(MT // 2):
        # bulk weight streams ride the gpsimd SW-DGE queue so their
        # slot-waits never block the sync queue's latency DMAs
        w2s = cx.stream.tile([128, 2, D], F8, name="w2s", tag="w2s")
        nc.gpsimd.dma_start(w2s[:], seg["w2p"].ap()[mjp])
        y1p = cx.stream.tile([128, 2, TOK], F8, name="y1p", tag="y1p")
        # both mj halves share one psM bank -> a single 512-wide gelu
        # (ACT_TABLE_LOAD is 1283ns; halving gelu count halves the thrash)
        ps = cx.psM.tile([128, 2 * TOK], F32, name="psm", tag="psm")
        for i in range(2):
            mj = 2 * mjp + i
            sl = slice(i * TOK, (i + 1) * TOK)
            for kdp in range(KT // 2):
                nc.tensor.matmul(ps[:, sl],
                                 w["g1"][:, mj, 2 * kdp:2 * kdp + 2, :],
                                 z2[:, 2 * kdp:2 * kdp + 2, :],
                                 start=(kdp == 0), stop=False, perf_mode=DR)
            nc.tensor.matmul(ps[:, sl], w["B2"][:, mj * 128:(mj + 1) * 128],
                             R2[:], start=False, stop=True)
        nc.scalar.activation(y1p[:, :, :], ps[:], AF.Gelu, scale=W8SI)
        for ft in range(KT):
            nc.tensor.matmul(ypsl(ft), w2s[:, :, ft * 128:(ft + 1) * 128],
                             y1p[:], start=(mjp == 0), stop=(mjp == MT // 2 - 1),
                             perf_mode=DR)

    if not combine:
        for ft in range(KT):
            yb = cx.tmp.tile([128, TOK], F32, name="yb", tag="yb")
            nc.vector.tensor_scalar(yb[:], ypsl(ft), W8SI,
                                    w["b2"][:, ft:ft + 1],
                                    op0=ALU.mult, op1=ALU.add)
            nc.vector.tensor_add(x232[:, ft, :], yb[:], x232[:, ft, :])
    else:
        yf = cx.act1.tile([128, KT, TOK], F32, name="yfld", tag="yfld")
        nc.gpsimd.dma_start(yf[:], yf_sc[t])
        for ft in range(KT):
            yb = cx.tmp.tile([128, TOK], F32, name="yb", tag="yb")
            nc.vector.tensor_scalar(yb[:], ypsl(ft), W8SI,
                                    w["b2"][:, ft:ft + 1],
                                    op0=ALU.mult, op1=ALU.add)
            nc.vector.tensor_add(yb[:], yb[:], yf[:, ft, :])
            nc.vector.tensor_add(x232[:, ft, :], yb[:], x232[:, ft, :])
    nc.gpsimd.dma_start(out_dst[t], x232[:])


# ---------------------------------------------------------------- entry point

@functools.cache
def _compiled_nc():
    return build_nc()


def kernel(**inputs):
    inputs = {k: np.asarray(v) for k, v in inputs.items()}
    nc = _compiled_nc()
    in_maps = make_in_maps(inputs)
    res = run_bass_kernel_spmd(nc, in_maps, list(range(NCORES)))
    return unshard_output(res.results)
